# revision 12
# baseline (speedup 1.0000x reference)
"""Causal self-attention (B=4, T=2048, C=768, 12 heads) on 8 Trainium2 NeuronCores.

Sharding: core c -> batch b = c//2, head group hg = c%2 (6 heads each).
Each core computes QKV for its (b, 6 heads), flash-style causal attention in a
transposed layout (scores^T = K^T-chunk @ Q, exp on ACT, ones-column in V for
row sums via the PE), transposes the per-head attention output on the PE, and
applies its slice of the output projection. Host sums the two half-head partial
y's per batch and adds b_proj.

Schedule: hp-major attention pipeline. Attention scores for head-pair hp start
as soon as q/k for hp0 token-chunk 0 exist; the remaining QKV matmuls (other
q/k chunks + all of V) are issued as PE fillers interleaved into the EXP-paced
scores loop so the PE never starves while ACT works through the exps. AV for
(hp,qt) lags one step behind scores (lag-1 software pipeline across hp
boundaries); projection chases AV during hp2. Diagonal-supertile score matmuls
are trimmed to the causal region.
"""

import numpy as np

import concourse.bacc as bacc
import concourse.bass as bass
import concourse.mybir as mybir
import concourse.tile as tile

# problem constants (hardcoded per harness contract)
B, T, C = 4, 2048, 768
NH, HD = 12, 64
NHL = 6            # heads per core
N_CORES = 8
TQ = 512           # q supertile width
NTB = T // 128     # 16 token blocks

F32 = mybir.dt.float32


def _build_program(cdt=mybir.dt.bfloat16, n_iters=1, with_bqk=False, with_bv=False):
    """Build the SPMD single-core program. Returns nc."""
    nc = bacc.Bacc("TRN2", target_bir_lowering=False, debug=False,
                   num_devices=N_CORES)

    xT_d = nc.dram_tensor("xT", [C, T], cdt, kind="ExternalInput")
    wqk_d = nc.dram_tensor("wqk", [C, 768], cdt, kind="ExternalInput")
    wv_d = nc.dram_tensor("wv", [C, 384], cdt, kind="ExternalInput")
    wp_d = nc.dram_tensor("wp", [384, C], cdt, kind="ExternalInput")
    mask_d = nc.dram_tensor("mask", [128, 128], cdt, kind="ExternalInput")
    id_d = nc.dram_tensor("ident", [128, 128], cdt, kind="ExternalInput")
    if with_bqk:
        bqk_d = nc.dram_tensor("bqk", [C, 1], F32, kind="ExternalInput")
    if with_bv:
        bv_d = nc.dram_tensor("bv", [128, 384], cdt, kind="ExternalInput")
    y_d = nc.dram_tensor("y", [T, C], F32, kind="ExternalOutput")

    with tile.TileContext(nc) as tc:
        with (
            tc.tile_pool(name="consts", bufs=1) as consts,
            tc.tile_pool(name="big", bufs=1) as big,
            tc.tile_pool(name="work", bufs=3) as work,
            tc.tile_pool(name="expool", bufs=30) as expool,
            tc.tile_pool(name="attp", bufs=3) as attp,
            tc.tile_pool(name="small", bufs=8) as small,
            tc.tile_pool(name="ps", bufs=3, space="PSUM") as psp,
        ):
            xT = consts.tile([128, 6, T], cdt)
            wqk = consts.tile([128, 6, 768], cdt)
            wv = consts.tile([128, 6, 384], cdt)
            wp = consts.tile([128, 3, 768], cdt)
            mask = consts.tile([128, 128], cdt)
            ident = consts.tile([128, 128], cdt)
            if with_bqk:
                bqk = consts.tile([128, 6, 1], F32)
            if with_bv:
                bv = consts.tile([128, 384], cdt)

            qt_sb = big.tile([128, 3, T], cdt)
            kt_sb = big.tile([128, 3, T], cdt)
            v_sb = big.tile([128, NTB, NHL, 65], cdt)
            attT_sb = big.tile([128, 3, T], cdt)

            def qk_nt(j, nt):
                """One 512-token chunk of q (j<3) / k (j>=3), partition chunk j%3."""
                dst = qt_sb if j < 3 else kt_sb
                jj = j % 3
                ps = psp.tile([128, 2, 512], F32, tag="A", name="qk")
                for kc in range(6):
                    nc.tensor.matmul(
                        ps[:, 0, :],
                        lhsT=wqk[:, kc, j * 128:(j + 1) * 128],
                        rhs=xT[:, kc, nt * 512:(nt + 1) * 512],
                        start=(kc == 0), stop=(kc == 5),
                    )
                if with_bqk:
                    nc.vector.tensor_scalar_add(
                        dst[:, jj, nt * 512:(nt + 1) * 512], ps[:, 0, :],
                        bqk[:, jj if j < 3 else jj + 3])
                else:
                    nc.vector.tensor_copy(
                        dst[:, jj, nt * 512:(nt + 1) * 512], ps[:, 0, :])

            def v_tb(tb):
                """V projection for one 128-token block, natural layout."""
                ps = psp.tile([128, 2, 512], F32, tag="A", name="psv")
                for kc in range(6):
                    nc.tensor.matmul(
                        ps[:, 0, 0:384],
                        lhsT=xT[:, kc, tb * 128:(tb + 1) * 128],
                        rhs=wv[:, kc, :],
                        start=(kc == 0), stop=(kc == 5),
                    )
                psr = ps[:, 0, 0:384].rearrange("p (h d) -> p h d", h=NHL)
                if with_bv:
                    nc.scalar.activation(
                        out=v_sb[:, tb, :, 0:HD], in_=psr,
                        func=mybir.ActivationFunctionType.Identity,
                        bias=0.0, scale=1.0)
                    nc.vector.tensor_add(
                        v_sb[:, tb, :, 0:HD], v_sb[:, tb, :, 0:HD],
                        bv.rearrange("p (h d) -> p h d", h=NHL))
                else:
                    nc.vector.tensor_copy(v_sb[:, tb, :, 0:HD], psr)

            def do_av(hp, qt, exs):
                """Normalized attention for (hp, qt) + transpose into attT_sb."""
                nkb = 4 * qt + 4
                att = attp.tile([128, 4, 2, HD], cdt, name="att")
                for h01 in range(2):
                    h = 2 * hp + h01
                    op = psp.tile([128, 4, 65], F32, tag="B", name="op", bufs=2)
                    for qq in range(4):
                        kbs = [kb for kb in range(nkb) if kb - 4 * qt <= qq]
                        for idx, kb in enumerate(kbs):
                            nc.tensor.matmul(
                                op[:, qq, :],
                                lhsT=exs[kb][:, h01, qq * 128:(qq + 1) * 128],
                                rhs=v_sb[:, kb, h, :],
                                start=(idx == 0), stop=(idx == len(kbs) - 1),
                            )
                    rc = small.tile([128, 4], F32, tag="rc")
                    nc.vector.reciprocal(rc[:], op[:, :, HD])
                    for qq in range(4):
                        nc.vector.tensor_scalar_mul(
                            att[:, qq, h01, :], op[:, qq, 0:HD], rc[:, qq:qq + 1])
                for qq in range(4):
                    qb = qt * 4 + qq
                    tp = psp.tile([128, 4, 65], cdt, tag="B", name="tp", bufs=2)
                    tpa = tp.rearrange("p a b -> p (a b)")[:, 0:128]
                    nc.tensor.transpose(tpa, att[:, qq, :, :], ident[:])
                    nc.vector.tensor_copy(
                        attT_sb[:, hp, qb * 128:(qb + 1) * 128], tpa)

            def do_proj(qt):
                """Output projection + store for qt's 4 token blocks."""
                for tb in range(qt * 4, qt * 4 + 4):
                    ps = psp.tile([128, 2, 512], F32, tag="A", name="pp")
                    for hp2 in range(3):
                        nc.tensor.matmul(
                            ps[:, 0, :],
                            lhsT=attT_sb[:, hp2, tb * 128:(tb + 1) * 128],
                            rhs=wp[:, hp2, 0:512],
                            start=(hp2 == 0), stop=(hp2 == 2))
                        nc.tensor.matmul(
                            ps[:, 1, 0:256],
                            lhsT=attT_sb[:, hp2, tb * 128:(tb + 1) * 128],
                            rhs=wp[:, hp2, 512:768],
                            start=(hp2 == 0), stop=(hp2 == 2))
                    ysb = work.tile([128, 768], F32, tag="ysb")
                    nc.vector.tensor_copy(ysb[:, 0:512], ps[:, 0, :])
                    nc.vector.tensor_copy(ysb[:, 512:768], ps[:, 1, 0:256])
                    nc.sync.dma_start(
                        y_d[tb * 128:(tb + 1) * 128, :], ysb[:])

            xT_r = xT_d.rearrange("(n p) t -> p n t", p=128)
            wqk_r = wqk_d.rearrange("(n p) m -> p n m", p=128)
            wv_r = wv_d.rearrange("(n p) m -> p n m", p=128)
            wp_r = wp_d.rearrange("(n p) m -> p n m", p=128)

            def load_main_inputs():
                for kc in range(6):
                    nc.sync.dma_start(xT[:, kc, :], xT_r[:, kc, :])
                    nc.sync.dma_start(wqk[:, kc, :], wqk_r[:, kc, :])
                for kc in range(6):
                    nc.sync.dma_start(wv[:, kc, :], wv_r[:, kc, :])
                if with_bqk:
                    nc.sync.dma_start(bqk[:], bqk_d.rearrange("(n p) o -> p n o", p=128))
                if with_bv:
                    nc.sync.dma_start(bv[:], bv_d[:])

            def prologue():
                load_main_inputs()
                nc.sync.dma_start(mask[:], mask_d[:])
                nc.sync.dma_start(ident[:], id_d[:])
                nc.sync.dma_start(wp[:], wp_r)
                nc.gpsimd.memset(v_sb[:, :, :, HD:HD + 1], 1.0)

            def body(reload=False):
                # lead-in: q/k for head-pair 0, token chunk 0 only
                qk_nt(0, 0)
                qk_nt(3, 0)

                # PE filler work, interleaved into the scores loop one item
                # per kb-block. hp0 carries the work that must land before
                # hp1 (q0/k0 tail chunks, all of V, q1/k1); hp1 carries
                # q2/k2 (needed by hp2 scores) paced evenly so the PE keeps
                # feeding during the ACT-bound middle of the iteration.
                hp_fillers = {0: [], 1: [], 2: []}
                hp_fillers[0] += [("qk", 0, 1), ("qk", 3, 1),
                                  ("v", 0), ("v", 1), ("v", 2), ("v", 3),
                                  ("qk", 0, 2), ("qk", 3, 2),
                                  ("qk", 0, 3), ("qk", 3, 3)]
                for tb in range(4, NTB):
                    hp_fillers[0].append(("v", tb))
                for j in (1, 4):
                    for nt in range(4):
                        hp_fillers[0].append(("qk", j, nt))
                for j in (2, 5):
                    for nt in range(4):
                        hp_fillers[1].append(("qk", j, nt))
                if reload:
                    # Re-issue next iteration's input loads right after the
                    # last reads of xT/wqk/wv. The SP stream stays ahead of
                    # the y-output DMAs so reloads overlap the attention tail
                    # instead of stalling the next iteration's start.
                    hp_fillers[1].append(("reload",))
                fill_state = {hp: [0, 0] for hp in range(3)}  # [popped, slot]

                def pop_filler(hp):
                    """Advance one kb slot for this hp and issue due fillers.
                    hp0: one per slot (dependency-ordered: V blocks must land
                    before their AV, q/k chunks before their scores). hp1:
                    paced evenly over its 40 kb slots."""
                    fl = hp_fillers[hp]
                    st = fill_state[hp]
                    st[1] += 1
                    if hp == 0:
                        want = min(len(fl), st[1])
                    else:
                        want = min(len(fl), (st[1] * len(fl) + 39) // 40)
                    while st[0] < want:
                        f = fl[st[0]]
                        st[0] += 1
                        if f[0] == "qk":
                            qk_nt(f[1], f[2])
                        elif f[0] == "v":
                            v_tb(f[1])
                        else:
                            load_main_inputs()

                seq = [(hp, qt) for hp in range(3) for qt in range(4)]
                prev = None          # (hp, qt, exs) pending AV
                for hp, qt in seq:
                    nkb = 4 * qt + 4
                    exs = []
                    for kb in range(nkb):
                        diag = kb - 4 * qt
                        q_lo = diag * 128 if diag >= 0 else 0
                        sc = psp.tile([128, 2, 512], F32, tag="A", name="sc")
                        for h01 in range(2):
                            pb = h01 * 64
                            nc.tensor.matmul(
                                sc[:, h01, q_lo:512],
                                lhsT=kt_sb[pb:pb + 64, hp, kb * 128:(kb + 1) * 128],
                                rhs=qt_sb[pb:pb + 64, hp, qt * 512 + q_lo:(qt + 1) * 512],
                                start=True, stop=True,
                            )
                        ex = expool.tile([128, 2, 512], cdt, tag="ex", name="ex")
                        nc.scalar.activation(
                            out=ex[:, :, q_lo:512], in_=sc[:, :, q_lo:512],
                            func=mybir.ActivationFunctionType.Exp)
                        if diag >= 0:
                            for h01 in range(2):
                                nc.gpsimd.tensor_mul(
                                    ex[:, h01, q_lo:q_lo + 128],
                                    ex[:, h01, q_lo:q_lo + 128],
                                    mask[:])
                        exs.append(ex)
                        pop_filler(hp)
                    if prev is not None:
                        phh, pqt, pexs = prev
                        do_av(phh, pqt, pexs)
                        if phh == 2:
                            do_proj(pqt)
                    prev = (hp, qt, exs)
                # drain any un-issued fillers (safety net; normally empty)
                for hp in range(3):
                    while fill_state[hp][0] < len(hp_fillers[hp]):
                        pop_filler(hp)
                phh, pqt, pexs = prev
                do_av(phh, pqt, pexs)
                do_proj(pqt)
                if reload:
                    # wp is only dead after the final proj; reload it behind
                    # the y-output DMAs so they are not held up.
                    nc.sync.dma_start(wp[:], wp_r)

            prologue()
            if n_iters == 1:
                body(reload=False)
            else:
                # unroll 2 bodies per hardware-loop iteration to halve the
                # loop back-edge cost (barrier + sequencer re-entry ~10us)
                n_loop, n_extra = divmod(n_iters, 2)
                with tc.For_i(0, n_loop, 1,
                              staggered_reset=True,
                              hint_engines=(mybir.EngineType.PE,
                                            mybir.EngineType.DVE,
                                            mybir.EngineType.Activation)):
                    body(reload=True)
                    body(reload=True)
                for _ in range(n_extra):
                    body(reload=True)

    nc.compile()
    return nc


def _host_prep(inputs, cdt_np):
    """Per-core input maps from full inputs."""
    x = np.asarray(inputs["x"], np.float32)
    w_attn = np.asarray(inputs["w_attn"], np.float32)
    b_attn = np.asarray(inputs["b_attn"], np.float32)
    w_proj = np.asarray(inputs["w_proj"], np.float32)

    mask = (np.arange(128)[:, None] <= np.arange(128)[None, :]).astype(cdt_np)
    ident = np.eye(128, dtype=cdt_np)
    with_bqk = bool(np.any(b_attn[0:1536] != 0))
    with_bv = bool(np.any(b_attn[1536:2304] != 0))

    in_maps = []
    for c in range(N_CORES):
        b, hg = c // 2, c % 2
        cols = slice(hg * 384, hg * 384 + 384)
        wq = w_attn[:, 0:768][:, cols] * 0.125
        wk = w_attn[:, 768:1536][:, cols]
        m = {
            "xT": np.ascontiguousarray(x[b].T).astype(cdt_np),
            "wqk": np.concatenate([wq, wk], axis=1).astype(cdt_np),
            "wv": np.ascontiguousarray(w_attn[:, 1536:2304][:, cols]).astype(cdt_np),
            "wp": np.ascontiguousarray(w_proj[cols, :]).astype(cdt_np),
            "mask": mask,
            "ident": ident,
        }
        if with_bqk:
            bq = b_attn[0:768][cols] * 0.125
            bk = b_attn[768:1536][cols]
            m["bqk"] = np.concatenate([bq, bk]).astype(np.float32).reshape(C, 1)
        if with_bv:
            bv = b_attn[1536:2304][cols].astype(cdt_np)
            m["bv"] = np.broadcast_to(bv, (128, 384)).copy()
        in_maps.append(m)
    return in_maps, with_bqk, with_bv


_CACHE = {}


def _get_runner(cdt, n_iters, with_bqk, with_bv, donate=True):
    """Build program + persistent jitted PJRT callable (cached)."""
    key = (str(cdt), n_iters, with_bqk, with_bv, donate)
    if key in _CACHE:
        return _CACHE[key]

    import jax
    from jax.sharding import Mesh, PartitionSpec
    from jax.experimental.shard_map import shard_map
    from concourse.bass2jax import (_bass_exec_p, install_neuronx_cc_hook,
                                    partition_id_tensor)

    nc = _build_program(cdt=cdt, n_iters=n_iters,
                        with_bqk=with_bqk, with_bv=with_bv)
    install_neuronx_cc_hook()

    partition_name = nc.partition_id_tensor.name if nc.partition_id_tensor else None
    in_names, out_names, out_avals = [], [], []
    for alloc in nc.m.functions[0].allocations:
        if not isinstance(alloc, mybir.MemoryLocationSet):
            continue
        name = alloc.memorylocations[0].name
        if alloc.kind == "ExternalInput":
            if name != partition_name:
                in_names.append(name)
        elif alloc.kind == "ExternalOutput":
            out_names.append(name)
            out_avals.append(jax.core.ShapedArray(
                tuple(alloc.tensor_shape), mybir.dt.np(alloc.dtype)))
    n_params = len(in_names)
    n_outs = len(out_avals)
    all_names = list(in_names) + list(out_names)
    if partition_name is not None:
        all_names.append(partition_name)
    donate_ = tuple(range(n_params, n_params + n_outs))

    def _bodyfn(*args):
        operands = list(args)
        if partition_name is not None:
            operands.append(partition_id_tensor())
        outs = _bass_exec_p.bind(
            *operands,
            out_avals=tuple(out_avals),
            in_names=tuple(all_names),
            out_names=tuple(out_names),
            lowering_input_output_aliases=(),
            sim_require_finite=True,
            sim_require_nnan=True,
            nc=nc,
        )
        return tuple(outs)

    devices = jax.devices()[:N_CORES]
    mesh = Mesh(np.asarray(devices), ("core",))
    in_specs = (PartitionSpec("core"),) * (n_params + n_outs)
    out_specs = (PartitionSpec("core"),) * n_outs
    fn = jax.jit(
        shard_map(_bodyfn, mesh=mesh, in_specs=in_specs, out_specs=out_specs,
                  check_rep=False),
        donate_argnums=donate_ if donate else (), keep_unused=True)

    runner = (fn, in_names, out_names, out_avals)
    _CACHE[key] = runner
    return runner


def _run(in_maps, cdt, n_iters, with_bqk, with_bv):
    import jax
    fn, in_names, out_names, out_avals = _get_runner(cdt, n_iters, with_bqk, with_bv)
    concat_in = [np.concatenate([m[nm] for m in in_maps], axis=0)
                 for nm in in_names]
    zeros = [np.zeros((N_CORES * av.shape[0], *av.shape[1:]), av.dtype)
             for av in out_avals]
    outs = fn(*concat_in, *zeros)
    jax.block_until_ready(outs)
    y = np.asarray(outs[out_names.index("y")]).reshape(N_CORES, T, C)
    return y


def kernel(**inputs) -> np.ndarray:
    import ml_dtypes
    cdt, cdt_np = mybir.dt.bfloat16, ml_dtypes.bfloat16
    in_maps, with_bqk, with_bv = _host_prep(inputs, cdt_np)
    y_parts = _run(in_maps, cdt, 1, with_bqk, with_bv)

    b_proj = np.asarray(inputs["b_proj"], np.float32)
    out = np.empty((B, T, C), np.float32)
    for b in range(B):
        out[b] = y_parts[2 * b] + y_parts[2 * b + 1] + b_proj
    return out


# revision 15
# speedup vs baseline: 1.1731x; 1.1731x over previous
"""Causal self-attention (B=4, T=2048, C=768, 12 heads) on 8 Trainium2 NeuronCores.

Sharding: core c -> batch b = c//2, head group hg = c%2 (6 heads each).
Each core computes QKV for its (b, 6 heads), flash-style causal attention in a
transposed layout (scores^T = K^T-chunk @ Q, exp on ACT, ones-column in V for
row sums via the PE), transposes the per-head attention output on the PE, and
applies its slice of the output projection. Host sums the two half-head partial
y's per batch and adds b_proj.

Schedule: hp-major attention pipeline. Attention scores for head-pair hp start
as soon as q/k for hp0 token-chunk 0 exist; the remaining QKV matmuls (other
q/k chunks + all of V) are issued as PE fillers interleaved into the EXP-paced
scores loop so the PE never starves while ACT works through the exps. AV for
(hp,qt) lags one step behind scores (lag-1 software pipeline across hp
boundaries); projection chases AV during hp2. Diagonal-supertile score matmuls
are trimmed to the causal region.
"""

import numpy as np

import concourse.bacc as bacc
import concourse.bass as bass
import concourse.mybir as mybir
import concourse.tile as tile

# problem constants (hardcoded per harness contract)
B, T, C = 4, 2048, 768
NH, HD = 12, 64
NHL = 6            # heads per core
N_CORES = 8
TQ = 512           # q supertile width
NTB = T // 128     # 16 token blocks

F32 = mybir.dt.float32


def _build_program(cdt=mybir.dt.bfloat16, n_iters=1, with_bqk=False, with_bv=False):
    """Build the SPMD single-core program. Returns nc."""
    nc = bacc.Bacc("TRN2", target_bir_lowering=False, debug=False,
                   num_devices=N_CORES)

    xT_d = nc.dram_tensor("xT", [C, T], cdt, kind="ExternalInput")
    wqk_d = nc.dram_tensor("wqk", [C, 768], cdt, kind="ExternalInput")
    wv_d = nc.dram_tensor("wv", [C, 384], cdt, kind="ExternalInput")
    wp_d = nc.dram_tensor("wp", [384, C], cdt, kind="ExternalInput")
    mask_d = nc.dram_tensor("mask", [128, 128], cdt, kind="ExternalInput")
    id_d = nc.dram_tensor("ident", [128, 128], cdt, kind="ExternalInput")
    if with_bqk:
        bqk_d = nc.dram_tensor("bqk", [C, 1], F32, kind="ExternalInput")
    if with_bv:
        bv_d = nc.dram_tensor("bv", [128, 384], cdt, kind="ExternalInput")
    y_d = nc.dram_tensor("y", [T, C], F32, kind="ExternalOutput")

    with tile.TileContext(nc) as tc:
        with (
            tc.tile_pool(name="consts", bufs=1) as consts,
            tc.tile_pool(name="big", bufs=1) as big,
            tc.tile_pool(name="work", bufs=3) as work,
            tc.tile_pool(name="expool", bufs=34) as expool,
            tc.tile_pool(name="attp", bufs=3) as attp,
            tc.tile_pool(name="small", bufs=8) as small,
            tc.tile_pool(name="ps", bufs=3, space="PSUM") as psp,
        ):
            xT = consts.tile([128, 6, T], cdt)
            wqk = consts.tile([128, 6, 768], cdt)
            wv = consts.tile([128, 6, 384], cdt)
            wp = consts.tile([128, 3, 768], cdt)
            mask = consts.tile([128, 128], cdt)
            ident = consts.tile([128, 128], cdt)
            if with_bqk:
                bqk = consts.tile([128, 6, 1], F32)
            if with_bv:
                bv = consts.tile([128, 384], cdt)

            qt_sb = big.tile([128, 3, T], cdt)
            kt_sb = big.tile([128, 3, T], cdt)
            v_sb = big.tile([128, NTB, NHL, 65], cdt)
            attT_sb = big.tile([128, 3, T], cdt)

            def qk_nt(j, nt):
                """One 512-token chunk of q (j<3) / k (j>=3), partition chunk j%3."""
                dst = qt_sb if j < 3 else kt_sb
                jj = j % 3
                ps = psp.tile([128, 2, 512], F32, tag="A", name="qk")
                for kc in range(6):
                    nc.tensor.matmul(
                        ps[:, 0, :],
                        lhsT=wqk[:, kc, j * 128:(j + 1) * 128],
                        rhs=xT[:, kc, nt * 512:(nt + 1) * 512],
                        start=(kc == 0), stop=(kc == 5),
                    )
                if with_bqk:
                    nc.vector.tensor_scalar_add(
                        dst[:, jj, nt * 512:(nt + 1) * 512], ps[:, 0, :],
                        bqk[:, jj if j < 3 else jj + 3])
                else:
                    nc.vector.tensor_copy(
                        dst[:, jj, nt * 512:(nt + 1) * 512], ps[:, 0, :])

            def v_tb(tb):
                """V projection for one 128-token block, natural layout."""
                ps = psp.tile([128, 2, 512], F32, tag="A", name="psv")
                for kc in range(6):
                    nc.tensor.matmul(
                        ps[:, 0, 0:384],
                        lhsT=xT[:, kc, tb * 128:(tb + 1) * 128],
                        rhs=wv[:, kc, :],
                        start=(kc == 0), stop=(kc == 5),
                    )
                psr = ps[:, 0, 0:384].rearrange("p (h d) -> p h d", h=NHL)
                if with_bv:
                    nc.scalar.activation(
                        out=v_sb[:, tb, :, 0:HD], in_=psr,
                        func=mybir.ActivationFunctionType.Identity,
                        bias=0.0, scale=1.0)
                    nc.vector.tensor_add(
                        v_sb[:, tb, :, 0:HD], v_sb[:, tb, :, 0:HD],
                        bv.rearrange("p (h d) -> p h d", h=NHL))
                else:
                    nc.vector.tensor_copy(v_sb[:, tb, :, 0:HD], psr)

            def av_items(hp, qt, exs):
                """Work-queue items for normalized attention of (hp, qt):
                8 op-accumulation groups (h01 x qq), then one transpose item.
                FIFO order guarantees h01's rescale runs before transposes."""
                nkb = 4 * qt + 4
                state = {}

                def grp(h01, qq):
                    def run():
                        if "att" not in state:
                            state["att"] = attp.tile([128, 4, 2, HD], cdt,
                                                     name="att")
                        if h01 not in state:
                            state[h01] = psp.tile([128, 4, 65], F32, tag="B",
                                                  name="op", bufs=2)
                        op = state[h01]
                        h = 2 * hp + h01
                        kbs = [kb for kb in range(nkb) if kb - 4 * qt <= qq]
                        for idx, kb in enumerate(kbs):
                            nc.tensor.matmul(
                                op[:, qq, :],
                                lhsT=exs[kb][:, h01, qq * 128:(qq + 1) * 128],
                                rhs=v_sb[:, kb, h, :],
                                start=(idx == 0), stop=(idx == len(kbs) - 1),
                            )
                        if qq == 3:
                            rc = small.tile([128, 4], F32, tag="rc")
                            nc.vector.reciprocal(rc[:], op[:, :, HD])
                            for q2 in range(4):
                                nc.vector.tensor_scalar_mul(
                                    state["att"][:, q2, h01, :],
                                    op[:, q2, 0:HD], rc[:, q2:q2 + 1])
                    return run

                def transp():
                    att = state["att"]
                    for qq in range(4):
                        qb = qt * 4 + qq
                        tp = psp.tile([128, 4, 65], cdt, tag="B", name="tp",
                                      bufs=2)
                        tpa = tp.rearrange("p a b -> p (a b)")[:, 0:128]
                        nc.tensor.transpose(tpa, att[:, qq, :, :], ident[:])
                        nc.vector.tensor_copy(
                            attT_sb[:, hp, qb * 128:(qb + 1) * 128], tpa)

                return [grp(h, q) for h in range(2) for q in range(4)] + [transp]

            def proj_items(qt):
                """Work-queue items: output projection + store, one per
                128-token block."""
                def one(tb):
                    def run():
                        ps = psp.tile([128, 2, 512], F32, tag="A", name="pp")
                        for hp2 in range(3):
                            nc.tensor.matmul(
                                ps[:, 0, :],
                                lhsT=attT_sb[:, hp2, tb * 128:(tb + 1) * 128],
                                rhs=wp[:, hp2, 0:512],
                                start=(hp2 == 0), stop=(hp2 == 2))
                            nc.tensor.matmul(
                                ps[:, 1, 0:256],
                                lhsT=attT_sb[:, hp2, tb * 128:(tb + 1) * 128],
                                rhs=wp[:, hp2, 512:768],
                                start=(hp2 == 0), stop=(hp2 == 2))
                        ysb = work.tile([128, 768], F32, tag="ysb")
                        nc.vector.tensor_copy(ysb[:, 0:512], ps[:, 0, :])
                        nc.vector.tensor_copy(ysb[:, 512:768], ps[:, 1, 0:256])
                        nc.sync.dma_start(
                            y_d[tb * 128:(tb + 1) * 128, :], ysb[:])
                    return run
                return [one(tb) for tb in range(qt * 4, qt * 4 + 4)]

            xT_r = xT_d.rearrange("(n p) t -> p n t", p=128)
            wqk_r = wqk_d.rearrange("(n p) m -> p n m", p=128)
            wv_r = wv_d.rearrange("(n p) m -> p n m", p=128)
            wp_r = wp_d.rearrange("(n p) m -> p n m", p=128)

            def load_main_inputs():
                for kc in range(6):
                    nc.sync.dma_start(xT[:, kc, :], xT_r[:, kc, :])
                    nc.sync.dma_start(wqk[:, kc, :], wqk_r[:, kc, :])
                for kc in range(6):
                    nc.sync.dma_start(wv[:, kc, :], wv_r[:, kc, :])
                if with_bqk:
                    nc.sync.dma_start(bqk[:], bqk_d.rearrange("(n p) o -> p n o", p=128))
                if with_bv:
                    nc.sync.dma_start(bv[:], bv_d[:])

            def prologue():
                load_main_inputs()
                nc.sync.dma_start(mask[:], mask_d[:])
                nc.sync.dma_start(ident[:], id_d[:])
                nc.sync.dma_start(wp[:], wp_r)
                nc.gpsimd.memset(v_sb[:, :, :, HD:HD + 1], 1.0)

            def body(reload=False):
                from collections import deque

                # lead-in: q/k for head-pair 0, token chunk 0 only
                qk_nt(0, 0)
                qk_nt(3, 0)

                def filler(f):
                    def run():
                        if f[0] == "qk":
                            qk_nt(f[1], f[2])
                        elif f[0] == "v":
                            v_tb(f[1])
                        else:
                            load_main_inputs()
                    return run

                # Deferred PE work (QKV chunks, V blocks, AV groups,
                # transposes, projection blocks, input reload) is held in one
                # FIFO queue and drained a few items per kb-slot so the PE
                # always has issued-and-ready work while ACT grinds through
                # the exps. Each filler carries (due_slot, deadline_position):
                # it enters the queue at its due slot (shaping the interleave)
                # and the deadline forces a drain before the position whose
                # scores/AV read its output (program-order dependency).
                # dues[hp] = list of (slot_within_hp, deadline_pos, spec)
                dues = {0: [], 1: [], 2: []}
                d0 = [(1, 1, ("qk", 0, 1)), (1, 1, ("qk", 3, 1)),
                      (2, 1, ("v", 0)), (2, 1, ("v", 1)),
                      (3, 1, ("v", 2)), (3, 1, ("v", 3)),
                      (4, 2, ("qk", 0, 2)), (5, 2, ("qk", 3, 2)),
                      (6, 3, ("qk", 0, 3)), (7, 3, ("qk", 3, 3))]
                for k, tb in enumerate(range(4, NTB)):
                    d0.append((8 + k, tb // 4 + 1, ("v", tb)))
                for k, (j, nt) in enumerate([(j, nt) for j in (1, 4)
                                             for nt in range(4)]):
                    d0.append((20 + k, 4, ("qk", j, nt)))
                dues[0] = d0
                d1 = [(1 + 4 * k, 8, ("qk", j, nt))
                      for k, (j, nt) in enumerate([(j, nt) for j in (2, 5)
                                                   for nt in range(4)])]
                if reload:
                    # Re-issue next iteration's input loads once xT/wqk/wv
                    # are dead; the SP stream stays ahead of the y-output
                    # DMAs so the reload overlaps the attention tail.
                    d1.append((34, 12, ("reload",)))
                dues[1] = d1

                pending = deque()  # of (deadline_pos, closure)

                def pop_one():
                    pending.popleft()[1]()

                seq = [(hp, qt) for hp in range(3) for qt in range(4)]
                hp_slot = {0: 0, 1: 0, 2: 0}
                prev = None          # (hp, qt, exs) pending AV
                for i, (hp, qt) in enumerate(seq):
                    # drain everything the upcoming reads depend on
                    while any(dl <= i for dl, _ in pending):
                        pop_one()
                    if prev is not None:
                        phh, pqt, pexs = prev
                        for it in av_items(phh, pqt, pexs):
                            pending.append((i + 2, it))
                        if phh == 2:
                            for it in proj_items(pqt):
                                pending.append((i + 2, it))
                    nkb = 4 * qt + 4
                    nkb_next = 4 * seq[i + 1][1] + 4 if i + 1 < len(seq) else 0
                    exs = []
                    for kb in range(nkb):
                        diag = kb - 4 * qt
                        q_lo = diag * 128 if diag >= 0 else 0
                        sc = psp.tile([128, 2, 512], F32, tag="A", name="sc")
                        for h01 in range(2):
                            pb = h01 * 64
                            nc.tensor.matmul(
                                sc[:, h01, q_lo:512],
                                lhsT=kt_sb[pb:pb + 64, hp, kb * 128:(kb + 1) * 128],
                                rhs=qt_sb[pb:pb + 64, hp, qt * 512 + q_lo:(qt + 1) * 512],
                                start=True, stop=True,
                            )
                        ex = expool.tile([128, 2, 512], cdt, tag="ex", name="ex")
                        nc.scalar.activation(
                            out=ex[:, :, q_lo:512], in_=sc[:, :, q_lo:512],
                            func=mybir.ActivationFunctionType.Exp)
                        if diag >= 0:
                            for h01 in range(2):
                                nc.gpsimd.tensor_mul(
                                    ex[:, h01, q_lo:q_lo + 128],
                                    ex[:, h01, q_lo:q_lo + 128],
                                    mask[:])
                        exs.append(ex)
                        # enqueue fillers coming due at this slot
                        hp_slot[hp] += 1
                        for slot, dl, spec in dues[hp]:
                            if slot == hp_slot[hp]:
                                pending.append((dl, filler(spec)))
                        # drain paced to finish by the end of the next position
                        horizon = (nkb - 1 - kb) + nkb_next + 1
                        budget = -(-len(pending) // horizon)
                        for _ in range(min(budget, len(pending))):
                            pop_one()
                    prev = (hp, qt, exs)
                # tail: last AV + projection, then any stragglers
                phh, pqt, pexs = prev
                for it in av_items(phh, pqt, pexs):
                    pending.append((99, it))
                for it in proj_items(pqt):
                    pending.append((99, it))
                while pending:
                    pop_one()
                if reload:
                    # wp is only dead after the final proj; reload it behind
                    # the y-output DMAs so they are not held up.
                    nc.sync.dma_start(wp[:], wp_r)

            prologue()
            if n_iters == 1:
                body(reload=False)
            else:
                # unroll 2 bodies per hardware-loop iteration to halve the
                # loop back-edge cost (barrier + sequencer re-entry ~10us)
                n_loop, n_extra = divmod(n_iters, 2)
                with tc.For_i(0, n_loop, 1,
                              staggered_reset=True,
                              hint_engines=(mybir.EngineType.PE,
                                            mybir.EngineType.DVE,
                                            mybir.EngineType.Activation)):
                    body(reload=True)
                    body(reload=True)
                for _ in range(n_extra):
                    body(reload=True)

    nc.compile()
    return nc


def _host_prep(inputs, cdt_np):
    """Per-core input maps from full inputs."""
    x = np.asarray(inputs["x"], np.float32)
    w_attn = np.asarray(inputs["w_attn"], np.float32)
    b_attn = np.asarray(inputs["b_attn"], np.float32)
    w_proj = np.asarray(inputs["w_proj"], np.float32)

    mask = (np.arange(128)[:, None] <= np.arange(128)[None, :]).astype(cdt_np)
    ident = np.eye(128, dtype=cdt_np)
    with_bqk = bool(np.any(b_attn[0:1536] != 0))
    with_bv = bool(np.any(b_attn[1536:2304] != 0))

    in_maps = []
    for c in range(N_CORES):
        b, hg = c // 2, c % 2
        cols = slice(hg * 384, hg * 384 + 384)
        wq = w_attn[:, 0:768][:, cols] * 0.125
        wk = w_attn[:, 768:1536][:, cols]
        m = {
            "xT": np.ascontiguousarray(x[b].T).astype(cdt_np),
            "wqk": np.concatenate([wq, wk], axis=1).astype(cdt_np),
            "wv": np.ascontiguousarray(w_attn[:, 1536:2304][:, cols]).astype(cdt_np),
            "wp": np.ascontiguousarray(w_proj[cols, :]).astype(cdt_np),
            "mask": mask,
            "ident": ident,
        }
        if with_bqk:
            bq = b_attn[0:768][cols] * 0.125
            bk = b_attn[768:1536][cols]
            m["bqk"] = np.concatenate([bq, bk]).astype(np.float32).reshape(C, 1)
        if with_bv:
            bv = b_attn[1536:2304][cols].astype(cdt_np)
            m["bv"] = np.broadcast_to(bv, (128, 384)).copy()
        in_maps.append(m)
    return in_maps, with_bqk, with_bv


_CACHE = {}


def _get_runner(cdt, n_iters, with_bqk, with_bv, donate=True):
    """Build program + persistent jitted PJRT callable (cached)."""
    key = (str(cdt), n_iters, with_bqk, with_bv, donate)
    if key in _CACHE:
        return _CACHE[key]

    import jax
    from jax.sharding import Mesh, PartitionSpec
    from jax.experimental.shard_map import shard_map
    from concourse.bass2jax import (_bass_exec_p, install_neuronx_cc_hook,
                                    partition_id_tensor)

    nc = _build_program(cdt=cdt, n_iters=n_iters,
                        with_bqk=with_bqk, with_bv=with_bv)
    install_neuronx_cc_hook()

    partition_name = nc.partition_id_tensor.name if nc.partition_id_tensor else None
    in_names, out_names, out_avals = [], [], []
    for alloc in nc.m.functions[0].allocations:
        if not isinstance(alloc, mybir.MemoryLocationSet):
            continue
        name = alloc.memorylocations[0].name
        if alloc.kind == "ExternalInput":
            if name != partition_name:
                in_names.append(name)
        elif alloc.kind == "ExternalOutput":
            out_names.append(name)
            out_avals.append(jax.core.ShapedArray(
                tuple(alloc.tensor_shape), mybir.dt.np(alloc.dtype)))
    n_params = len(in_names)
    n_outs = len(out_avals)
    all_names = list(in_names) + list(out_names)
    if partition_name is not None:
        all_names.append(partition_name)
    donate_ = tuple(range(n_params, n_params + n_outs))

    def _bodyfn(*args):
        operands = list(args)
        if partition_name is not None:
            operands.append(partition_id_tensor())
        outs = _bass_exec_p.bind(
            *operands,
            out_avals=tuple(out_avals),
            in_names=tuple(all_names),
            out_names=tuple(out_names),
            lowering_input_output_aliases=(),
            sim_require_finite=True,
            sim_require_nnan=True,
            nc=nc,
        )
        return tuple(outs)

    devices = jax.devices()[:N_CORES]
    mesh = Mesh(np.asarray(devices), ("core",))
    in_specs = (PartitionSpec("core"),) * (n_params + n_outs)
    out_specs = (PartitionSpec("core"),) * n_outs
    fn = jax.jit(
        shard_map(_bodyfn, mesh=mesh, in_specs=in_specs, out_specs=out_specs,
                  check_rep=False),
        donate_argnums=donate_ if donate else (), keep_unused=True)

    runner = (fn, in_names, out_names, out_avals)
    _CACHE[key] = runner
    return runner


def _run(in_maps, cdt, n_iters, with_bqk, with_bv):
    import jax
    fn, in_names, out_names, out_avals = _get_runner(cdt, n_iters, with_bqk, with_bv)
    concat_in = [np.concatenate([m[nm] for m in in_maps], axis=0)
                 for nm in in_names]
    zeros = [np.zeros((N_CORES * av.shape[0], *av.shape[1:]), av.dtype)
             for av in out_avals]
    outs = fn(*concat_in, *zeros)
    jax.block_until_ready(outs)
    y = np.asarray(outs[out_names.index("y")]).reshape(N_CORES, T, C)
    return y


def kernel(**inputs) -> np.ndarray:
    import ml_dtypes
    cdt, cdt_np = mybir.dt.bfloat16, ml_dtypes.bfloat16
    in_maps, with_bqk, with_bv = _host_prep(inputs, cdt_np)
    y_parts = _run(in_maps, cdt, 1, with_bqk, with_bv)

    b_proj = np.asarray(inputs["b_proj"], np.float32)
    out = np.empty((B, T, C), np.float32)
    for b in range(B):
        out[b] = y_parts[2 * b] + y_parts[2 * b + 1] + b_proj
    return out


# revision 17
# speedup vs baseline: 1.2002x; 1.0231x over previous
"""Causal self-attention (B=4, T=2048, C=768, 12 heads) on 8 Trainium2 NeuronCores.

Sharding: core c -> batch b = c//2, head group hg = c%2 (6 heads each).
Each core computes QKV for its (b, 6 heads), flash-style causal attention in a
transposed layout (scores^T = K^T-chunk @ Q, exp on ACT, ones-column in V for
row sums via the PE), transposes the per-head attention output on the PE, and
applies its slice of the output projection. Host sums the two half-head partial
y's per batch and adds b_proj.

Schedule: hp-major attention pipeline. Attention scores for head-pair hp start
as soon as q/k for hp0 token-chunk 0 exist; the remaining QKV matmuls (other
q/k chunks + all of V) are issued as PE fillers interleaved into the EXP-paced
scores loop so the PE never starves while ACT works through the exps. AV for
(hp,qt) lags one step behind scores (lag-1 software pipeline across hp
boundaries); projection chases AV during hp2. Diagonal-supertile score matmuls
are trimmed to the causal region.
"""

import numpy as np

import concourse.bacc as bacc
import concourse.bass as bass
import concourse.mybir as mybir
import concourse.tile as tile

# problem constants (hardcoded per harness contract)
B, T, C = 4, 2048, 768
NH, HD = 12, 64
NHL = 6            # heads per core
N_CORES = 8
TQ = 512           # q supertile width
NTB = T // 128     # 16 token blocks

F32 = mybir.dt.float32


def _build_program(cdt=mybir.dt.bfloat16, n_iters=1, with_bqk=False, with_bv=False):
    """Build the SPMD single-core program. Returns nc."""
    nc = bacc.Bacc("TRN2", target_bir_lowering=False, debug=False,
                   num_devices=N_CORES)

    xT_d = nc.dram_tensor("xT", [C, T], cdt, kind="ExternalInput")
    wqk_d = nc.dram_tensor("wqk", [C, 768], cdt, kind="ExternalInput")
    wv_d = nc.dram_tensor("wv", [C, 384], cdt, kind="ExternalInput")
    wp_d = nc.dram_tensor("wp", [384, C], cdt, kind="ExternalInput")
    mask_d = nc.dram_tensor("mask", [128, 128], cdt, kind="ExternalInput")
    id_d = nc.dram_tensor("ident", [128, 128], cdt, kind="ExternalInput")
    if with_bqk:
        bqk_d = nc.dram_tensor("bqk", [C, 1], F32, kind="ExternalInput")
    if with_bv:
        bv_d = nc.dram_tensor("bv", [128, 384], cdt, kind="ExternalInput")
    y_d = nc.dram_tensor("y", [T, C], F32, kind="ExternalOutput")

    with tile.TileContext(nc) as tc:
        with (
            tc.tile_pool(name="consts", bufs=1) as consts,
            tc.tile_pool(name="big", bufs=1) as big,
            tc.tile_pool(name="work", bufs=3) as work,
            tc.tile_pool(name="expool", bufs=34) as expool,
            tc.tile_pool(name="attp", bufs=3) as attp,
            tc.tile_pool(name="small", bufs=8) as small,
            tc.tile_pool(name="ps", bufs=3, space="PSUM") as psp,
        ):
            xT = consts.tile([128, 6, T], cdt)
            wqk = consts.tile([128, 6, 768], cdt)
            wv = consts.tile([128, 6, 384], cdt)
            wp = consts.tile([128, 3, 768], cdt)
            mask = consts.tile([128, 128], cdt)
            ident = consts.tile([128, 128], cdt)
            if with_bqk:
                bqk = consts.tile([128, 6, 1], F32)
            if with_bv:
                bv = consts.tile([128, 384], cdt)

            qt_sb = big.tile([128, 3, T], cdt)
            kt_sb = big.tile([128, 3, T], cdt)
            v_sb = big.tile([128, NTB, NHL, 65], cdt)
            attT_sb = big.tile([128, 3, T], cdt)

            def qk_nt(j, nt):
                """One 512-token chunk of q (j<3) / k (j>=3), partition chunk j%3."""
                dst = qt_sb if j < 3 else kt_sb
                jj = j % 3
                ps = psp.tile([128, 512], F32, tag="B", name="qk", bufs=2)
                for kc in range(6):
                    nc.tensor.matmul(
                        ps[:],
                        lhsT=wqk[:, kc, j * 128:(j + 1) * 128],
                        rhs=xT[:, kc, nt * 512:(nt + 1) * 512],
                        start=(kc == 0), stop=(kc == 5),
                    )
                if with_bqk:
                    nc.vector.tensor_scalar_add(
                        dst[:, jj, nt * 512:(nt + 1) * 512], ps[:],
                        bqk[:, jj if j < 3 else jj + 3])
                else:
                    nc.vector.tensor_copy(
                        dst[:, jj, nt * 512:(nt + 1) * 512], ps[:])

            def v_tb(tb):
                """V projection for one 128-token block, natural layout."""
                ps = psp.tile([128, 384], F32, tag="B", name="psv", bufs=2)
                for kc in range(6):
                    nc.tensor.matmul(
                        ps[:],
                        lhsT=xT[:, kc, tb * 128:(tb + 1) * 128],
                        rhs=wv[:, kc, :],
                        start=(kc == 0), stop=(kc == 5),
                    )
                psr = ps[:].rearrange("p (h d) -> p h d", h=NHL)
                if with_bv:
                    nc.scalar.activation(
                        out=v_sb[:, tb, :, 0:HD], in_=psr,
                        func=mybir.ActivationFunctionType.Identity,
                        bias=0.0, scale=1.0)
                    nc.vector.tensor_add(
                        v_sb[:, tb, :, 0:HD], v_sb[:, tb, :, 0:HD],
                        bv.rearrange("p (h d) -> p h d", h=NHL))
                else:
                    nc.vector.tensor_copy(v_sb[:, tb, :, 0:HD], psr)

            def av_items(hp, qt, exs):
                """Work-queue items for normalized attention of (hp, qt):
                one op-accumulation item per h01 (4 qq groups + rescale, so
                the op PSUM tile lives within a single item), then one
                transpose item. FIFO order keeps rescales before transposes."""
                nkb = 4 * qt + 4
                state = {}

                def havs(h01):
                    def run():
                        if "att" not in state:
                            state["att"] = attp.tile([128, 4, 2, HD], cdt,
                                                     name="att")
                        op = psp.tile([128, 4, 65], F32, tag="B",
                                      name="op", bufs=2)
                        h = 2 * hp + h01
                        for qq in range(4):
                            kbs = [kb for kb in range(nkb) if kb - 4 * qt <= qq]
                            for idx, kb in enumerate(kbs):
                                nc.tensor.matmul(
                                    op[:, qq, :],
                                    lhsT=exs[kb][:, h01, qq * 128:(qq + 1) * 128],
                                    rhs=v_sb[:, kb, h, :],
                                    start=(idx == 0), stop=(idx == len(kbs) - 1),
                                )
                        rc = small.tile([128, 4], F32, tag="rc")
                        nc.vector.reciprocal(rc[:], op[:, :, HD])
                        for q2 in range(4):
                            nc.vector.tensor_scalar_mul(
                                state["att"][:, q2, h01, :],
                                op[:, q2, 0:HD], rc[:, q2:q2 + 1])
                    return run

                def transp():
                    att = state["att"]
                    for qq in range(4):
                        qb = qt * 4 + qq
                        tp = psp.tile([128, 4, 65], cdt, tag="B", name="tp",
                                      bufs=2)
                        tpa = tp.rearrange("p a b -> p (a b)")[:, 0:128]
                        nc.tensor.transpose(tpa, att[:, qq, :, :], ident[:])
                        nc.vector.tensor_copy(
                            attT_sb[:, hp, qb * 128:(qb + 1) * 128], tpa)

                return [havs(0), havs(1), transp]

            def proj_items(qt):
                """Work-queue items: output projection + store, one per
                128-token block."""
                def one(tb):
                    def run():
                        pa = psp.tile([128, 512], F32, tag="B", name="pa",
                                      bufs=2)
                        pb = psp.tile([128, 256], F32, tag="B", name="pb",
                                      bufs=2)
                        for hp2 in range(3):
                            nc.tensor.matmul(
                                pa[:],
                                lhsT=attT_sb[:, hp2, tb * 128:(tb + 1) * 128],
                                rhs=wp[:, hp2, 0:512],
                                start=(hp2 == 0), stop=(hp2 == 2))
                            nc.tensor.matmul(
                                pb[:],
                                lhsT=attT_sb[:, hp2, tb * 128:(tb + 1) * 128],
                                rhs=wp[:, hp2, 512:768],
                                start=(hp2 == 0), stop=(hp2 == 2))
                        ysb = work.tile([128, 768], F32, tag="ysb")
                        nc.vector.tensor_copy(ysb[:, 0:512], pa[:])
                        nc.vector.tensor_copy(ysb[:, 512:768], pb[:])
                        nc.sync.dma_start(
                            y_d[tb * 128:(tb + 1) * 128, :], ysb[:])
                    return run
                return [one(tb) for tb in range(qt * 4, qt * 4 + 4)]

            xT_r = xT_d.rearrange("(n p) t -> p n t", p=128)
            wqk_r = wqk_d.rearrange("(n p) m -> p n m", p=128)
            wv_r = wv_d.rearrange("(n p) m -> p n m", p=128)
            wp_r = wp_d.rearrange("(n p) m -> p n m", p=128)

            def load_main_inputs():
                for kc in range(6):
                    nc.sync.dma_start(xT[:, kc, :], xT_r[:, kc, :])
                    nc.sync.dma_start(wqk[:, kc, :], wqk_r[:, kc, :])
                for kc in range(6):
                    nc.sync.dma_start(wv[:, kc, :], wv_r[:, kc, :])
                if with_bqk:
                    nc.sync.dma_start(bqk[:], bqk_d.rearrange("(n p) o -> p n o", p=128))
                if with_bv:
                    nc.sync.dma_start(bv[:], bv_d[:])

            def prologue():
                load_main_inputs()
                nc.sync.dma_start(mask[:], mask_d[:])
                nc.sync.dma_start(ident[:], id_d[:])
                nc.sync.dma_start(wp[:], wp_r)
                nc.gpsimd.memset(v_sb[:, :, :, HD:HD + 1], 1.0)

            def body(reload=False):
                from collections import deque

                # lead-in: q/k for head-pair 0, token chunk 0 only
                qk_nt(0, 0)
                qk_nt(3, 0)

                def filler(f):
                    def run():
                        if f[0] == "qk":
                            qk_nt(f[1], f[2])
                        elif f[0] == "v":
                            v_tb(f[1])
                        else:
                            load_main_inputs()
                    return run

                # Deferred PE work (QKV chunks, V blocks, AV groups,
                # transposes, projection blocks, input reload) is held in one
                # FIFO queue and drained a few items per kb-slot so the PE
                # always has issued-and-ready work while ACT grinds through
                # the exps. Each filler carries (due_slot, deadline_position):
                # it enters the queue at its due slot (shaping the interleave)
                # and the deadline forces a drain before the position whose
                # scores/AV read its output (program-order dependency).
                # dues[hp] = list of (slot_within_hp, deadline_pos, spec)
                dues = {0: [], 1: [], 2: []}
                d0 = [(1, 1, ("qk", 0, 1)), (1, 1, ("qk", 3, 1)),
                      (2, 1, ("v", 0)), (2, 1, ("v", 1)),
                      (3, 1, ("v", 2)), (3, 1, ("v", 3)),
                      (4, 2, ("qk", 0, 2)), (5, 2, ("qk", 3, 2)),
                      (6, 3, ("qk", 0, 3)), (7, 3, ("qk", 3, 3))]
                for k, tb in enumerate(range(4, NTB)):
                    d0.append((8 + k, tb // 4 + 1, ("v", tb)))
                for k, (j, nt) in enumerate([(j, nt) for j in (1, 4)
                                             for nt in range(4)]):
                    d0.append((20 + k, 4, ("qk", j, nt)))
                dues[0] = d0
                d1 = [(1 + 4 * k, 8, ("qk", j, nt))
                      for k, (j, nt) in enumerate([(j, nt) for j in (2, 5)
                                                   for nt in range(4)])]
                if reload:
                    # Re-issue next iteration's input loads once xT/wqk/wv
                    # are dead; the SP stream stays ahead of the y-output
                    # DMAs so the reload overlaps the attention tail.
                    d1.append((34, 12, ("reload",)))
                dues[1] = d1

                pending = deque()  # of (deadline_pos, closure)

                def pop_one():
                    pending.popleft()[1]()

                seq = [(hp, qt) for hp in range(3) for qt in range(4)]
                hp_slot = {0: 0, 1: 0, 2: 0}
                prev = None          # (hp, qt, exs) pending AV
                for i, (hp, qt) in enumerate(seq):
                    # drain everything the upcoming reads depend on
                    while any(dl <= i for dl, _ in pending):
                        pop_one()
                    if prev is not None:
                        phh, pqt, pexs = prev
                        for it in av_items(phh, pqt, pexs):
                            pending.append((i + 2, it))
                        if phh == 2:
                            for it in proj_items(pqt):
                                pending.append((i + 2, it))
                    nkb = 4 * qt + 4
                    nkb_next = 4 * seq[i + 1][1] + 4 if i + 1 < len(seq) else 0
                    exs = []
                    for kb in range(nkb):
                        diag = kb - 4 * qt
                        q_lo = diag * 128 if diag >= 0 else 0
                        sc = psp.tile([128, 2, 512], F32, tag="A", name="sc")
                        for h01 in range(2):
                            pb = h01 * 64
                            nc.tensor.matmul(
                                sc[:, h01, q_lo:512],
                                lhsT=kt_sb[pb:pb + 64, hp, kb * 128:(kb + 1) * 128],
                                rhs=qt_sb[pb:pb + 64, hp, qt * 512 + q_lo:(qt + 1) * 512],
                                start=True, stop=True,
                            )
                        ex = expool.tile([128, 2, 512], cdt, tag="ex", name="ex")
                        nc.scalar.activation(
                            out=ex[:, :, q_lo:512], in_=sc[:, :, q_lo:512],
                            func=mybir.ActivationFunctionType.Exp)
                        if diag >= 0:
                            for h01 in range(2):
                                nc.gpsimd.tensor_mul(
                                    ex[:, h01, q_lo:q_lo + 128],
                                    ex[:, h01, q_lo:q_lo + 128],
                                    mask[:])
                        exs.append(ex)
                        # enqueue fillers coming due at this slot
                        hp_slot[hp] += 1
                        for slot, dl, spec in dues[hp]:
                            if slot == hp_slot[hp]:
                                pending.append((dl, filler(spec)))
                        # drain paced to finish by the end of the next position
                        horizon = (nkb - 1 - kb) + nkb_next + 1
                        budget = -(-len(pending) // horizon)
                        for _ in range(min(budget, len(pending))):
                            pop_one()
                    prev = (hp, qt, exs)
                # tail: last AV + projection, then any stragglers
                phh, pqt, pexs = prev
                for it in av_items(phh, pqt, pexs):
                    pending.append((99, it))
                for it in proj_items(pqt):
                    pending.append((99, it))
                while pending:
                    pop_one()
                if reload:
                    # wp is only dead after the final proj; reload it behind
                    # the y-output DMAs so they are not held up.
                    nc.sync.dma_start(wp[:], wp_r)

            prologue()
            if n_iters == 1:
                body(reload=False)
            else:
                # unroll 2 bodies per hardware-loop iteration to halve the
                # loop back-edge cost (barrier + sequencer re-entry ~10us)
                n_loop, n_extra = divmod(n_iters, 2)
                with tc.For_i(0, n_loop, 1,
                              staggered_reset=True,
                              hint_engines=(mybir.EngineType.PE,
                                            mybir.EngineType.DVE,
                                            mybir.EngineType.Activation)):
                    body(reload=True)
                    body(reload=True)
                for _ in range(n_extra):
                    body(reload=True)

    nc.compile()
    return nc


def _host_prep(inputs, cdt_np):
    """Per-core input maps from full inputs."""
    x = np.asarray(inputs["x"], np.float32)
    w_attn = np.asarray(inputs["w_attn"], np.float32)
    b_attn = np.asarray(inputs["b_attn"], np.float32)
    w_proj = np.asarray(inputs["w_proj"], np.float32)

    mask = (np.arange(128)[:, None] <= np.arange(128)[None, :]).astype(cdt_np)
    ident = np.eye(128, dtype=cdt_np)
    with_bqk = bool(np.any(b_attn[0:1536] != 0))
    with_bv = bool(np.any(b_attn[1536:2304] != 0))

    in_maps = []
    for c in range(N_CORES):
        b, hg = c // 2, c % 2
        cols = slice(hg * 384, hg * 384 + 384)
        wq = w_attn[:, 0:768][:, cols] * 0.125
        wk = w_attn[:, 768:1536][:, cols]
        m = {
            "xT": np.ascontiguousarray(x[b].T).astype(cdt_np),
            "wqk": np.concatenate([wq, wk], axis=1).astype(cdt_np),
            "wv": np.ascontiguousarray(w_attn[:, 1536:2304][:, cols]).astype(cdt_np),
            "wp": np.ascontiguousarray(w_proj[cols, :]).astype(cdt_np),
            "mask": mask,
            "ident": ident,
        }
        if with_bqk:
            bq = b_attn[0:768][cols] * 0.125
            bk = b_attn[768:1536][cols]
            m["bqk"] = np.concatenate([bq, bk]).astype(np.float32).reshape(C, 1)
        if with_bv:
            bv = b_attn[1536:2304][cols].astype(cdt_np)
            m["bv"] = np.broadcast_to(bv, (128, 384)).copy()
        in_maps.append(m)
    return in_maps, with_bqk, with_bv


_CACHE = {}


def _get_runner(cdt, n_iters, with_bqk, with_bv, donate=True):
    """Build program + persistent jitted PJRT callable (cached)."""
    key = (str(cdt), n_iters, with_bqk, with_bv, donate)
    if key in _CACHE:
        return _CACHE[key]

    import jax
    from jax.sharding import Mesh, PartitionSpec
    from jax.experimental.shard_map import shard_map
    from concourse.bass2jax import (_bass_exec_p, install_neuronx_cc_hook,
                                    partition_id_tensor)

    nc = _build_program(cdt=cdt, n_iters=n_iters,
                        with_bqk=with_bqk, with_bv=with_bv)
    install_neuronx_cc_hook()

    partition_name = nc.partition_id_tensor.name if nc.partition_id_tensor else None
    in_names, out_names, out_avals = [], [], []
    for alloc in nc.m.functions[0].allocations:
        if not isinstance(alloc, mybir.MemoryLocationSet):
            continue
        name = alloc.memorylocations[0].name
        if alloc.kind == "ExternalInput":
            if name != partition_name:
                in_names.append(name)
        elif alloc.kind == "ExternalOutput":
            out_names.append(name)
            out_avals.append(jax.core.ShapedArray(
                tuple(alloc.tensor_shape), mybir.dt.np(alloc.dtype)))
    n_params = len(in_names)
    n_outs = len(out_avals)
    all_names = list(in_names) + list(out_names)
    if partition_name is not None:
        all_names.append(partition_name)
    donate_ = tuple(range(n_params, n_params + n_outs))

    def _bodyfn(*args):
        operands = list(args)
        if partition_name is not None:
            operands.append(partition_id_tensor())
        outs = _bass_exec_p.bind(
            *operands,
            out_avals=tuple(out_avals),
            in_names=tuple(all_names),
            out_names=tuple(out_names),
            lowering_input_output_aliases=(),
            sim_require_finite=True,
            sim_require_nnan=True,
            nc=nc,
        )
        return tuple(outs)

    devices = jax.devices()[:N_CORES]
    mesh = Mesh(np.asarray(devices), ("core",))
    in_specs = (PartitionSpec("core"),) * (n_params + n_outs)
    out_specs = (PartitionSpec("core"),) * n_outs
    fn = jax.jit(
        shard_map(_bodyfn, mesh=mesh, in_specs=in_specs, out_specs=out_specs,
                  check_rep=False),
        donate_argnums=donate_ if donate else (), keep_unused=True)

    runner = (fn, in_names, out_names, out_avals)
    _CACHE[key] = runner
    return runner


def _run(in_maps, cdt, n_iters, with_bqk, with_bv):
    import jax
    fn, in_names, out_names, out_avals = _get_runner(cdt, n_iters, with_bqk, with_bv)
    concat_in = [np.concatenate([m[nm] for m in in_maps], axis=0)
                 for nm in in_names]
    zeros = [np.zeros((N_CORES * av.shape[0], *av.shape[1:]), av.dtype)
             for av in out_avals]
    outs = fn(*concat_in, *zeros)
    jax.block_until_ready(outs)
    y = np.asarray(outs[out_names.index("y")]).reshape(N_CORES, T, C)
    return y


def kernel(**inputs) -> np.ndarray:
    import ml_dtypes
    cdt, cdt_np = mybir.dt.bfloat16, ml_dtypes.bfloat16
    in_maps, with_bqk, with_bv = _host_prep(inputs, cdt_np)
    y_parts = _run(in_maps, cdt, 1, with_bqk, with_bv)

    b_proj = np.asarray(inputs["b_proj"], np.float32)
    out = np.empty((B, T, C), np.float32)
    for b in range(B):
        out[b] = y_parts[2 * b] + y_parts[2 * b + 1] + b_proj
    return out


# revision 18
# speedup vs baseline: 1.2114x; 1.0093x over previous
"""Causal self-attention (B=4, T=2048, C=768, 12 heads) on 8 Trainium2 NeuronCores.

Sharding: core c -> batch b = c//2, head group hg = c%2 (6 heads each).
Each core computes QKV for its (b, 6 heads), flash-style causal attention in a
transposed layout (scores^T = K^T-chunk @ Q, exp on ACT, ones-column in V for
row sums via the PE), transposes the per-head attention output on the PE, and
applies its slice of the output projection. Host sums the two half-head partial
y's per batch and adds b_proj.

Schedule: hp-major attention pipeline. Attention scores for head-pair hp start
as soon as q/k for hp0 token-chunk 0 exist; the remaining QKV matmuls (other
q/k chunks + all of V) are issued as PE fillers interleaved into the EXP-paced
scores loop so the PE never starves while ACT works through the exps. AV for
(hp,qt) lags one step behind scores (lag-1 software pipeline across hp
boundaries); projection chases AV during hp2. Diagonal-supertile score matmuls
are trimmed to the causal region.
"""

import numpy as np

import concourse.bacc as bacc
import concourse.bass as bass
import concourse.mybir as mybir
import concourse.tile as tile

# problem constants (hardcoded per harness contract)
B, T, C = 4, 2048, 768
NH, HD = 12, 64
NHL = 6            # heads per core
N_CORES = 8
TQ = 512           # q supertile width
NTB = T // 128     # 16 token blocks

F32 = mybir.dt.float32


def _build_program(cdt=mybir.dt.bfloat16, n_iters=1, with_bqk=False, with_bv=False):
    """Build the SPMD single-core program. Returns nc."""
    nc = bacc.Bacc("TRN2", target_bir_lowering=False, debug=False,
                   num_devices=N_CORES)

    xT_d = nc.dram_tensor("xT", [C, T], cdt, kind="ExternalInput")
    wqk_d = nc.dram_tensor("wqk", [C, 768], cdt, kind="ExternalInput")
    wv_d = nc.dram_tensor("wv", [C, 384], cdt, kind="ExternalInput")
    wp_d = nc.dram_tensor("wp", [384, C], cdt, kind="ExternalInput")
    mask_d = nc.dram_tensor("mask", [128, 128], cdt, kind="ExternalInput")
    id_d = nc.dram_tensor("ident", [128, 128], cdt, kind="ExternalInput")
    if with_bqk:
        bqk_d = nc.dram_tensor("bqk", [C, 1], F32, kind="ExternalInput")
    if with_bv:
        bv_d = nc.dram_tensor("bv", [128, 384], cdt, kind="ExternalInput")
    y_d = nc.dram_tensor("y", [T, C], F32, kind="ExternalOutput")

    with tile.TileContext(nc) as tc:
        with (
            tc.tile_pool(name="consts", bufs=1) as consts,
            tc.tile_pool(name="big", bufs=1) as big,
            tc.tile_pool(name="work", bufs=3) as work,
            tc.tile_pool(name="expool", bufs=34) as expool,
            tc.tile_pool(name="attp", bufs=3) as attp,
            tc.tile_pool(name="small", bufs=8) as small,
            tc.tile_pool(name="ps", bufs=3, space="PSUM") as psp,
        ):
            xT = consts.tile([128, 6, T], cdt)
            wqk = consts.tile([128, 6, 768], cdt)
            wv = consts.tile([128, 6, 384], cdt)
            wp = consts.tile([128, 3, 768], cdt)
            mask = consts.tile([128, 128], cdt)
            ident = consts.tile([128, 128], cdt)
            if with_bqk:
                bqk = consts.tile([128, 6, 1], F32)
            if with_bv:
                bv = consts.tile([128, 384], cdt)

            qt_sb = big.tile([128, 3, T], cdt)
            kt_sb = big.tile([128, 3, T], cdt)
            v_sb = big.tile([128, NTB, NHL, 65], cdt)
            attT_sb = big.tile([128, 3, T], cdt)

            def qk_nt(j, nt):
                """One 512-token chunk of q (j<3) / k (j>=3), partition chunk j%3."""
                dst = qt_sb if j < 3 else kt_sb
                jj = j % 3
                ps = psp.tile([128, 512], F32, tag="B", name="qk", bufs=2)
                for kc in range(6):
                    nc.tensor.matmul(
                        ps[:],
                        lhsT=wqk[:, kc, j * 128:(j + 1) * 128],
                        rhs=xT[:, kc, nt * 512:(nt + 1) * 512],
                        start=(kc == 0), stop=(kc == 5),
                    )
                if with_bqk:
                    nc.vector.tensor_scalar_add(
                        dst[:, jj, nt * 512:(nt + 1) * 512], ps[:],
                        bqk[:, jj if j < 3 else jj + 3])
                else:
                    nc.vector.tensor_copy(
                        dst[:, jj, nt * 512:(nt + 1) * 512], ps[:])

            def v_tb(tb):
                """V projection for one 128-token block, natural layout."""
                ps = psp.tile([128, 384], F32, tag="B", name="psv", bufs=2)
                for kc in range(6):
                    nc.tensor.matmul(
                        ps[:],
                        lhsT=xT[:, kc, tb * 128:(tb + 1) * 128],
                        rhs=wv[:, kc, :],
                        start=(kc == 0), stop=(kc == 5),
                    )
                psr = ps[:].rearrange("p (h d) -> p h d", h=NHL)
                if with_bv:
                    nc.scalar.activation(
                        out=v_sb[:, tb, :, 0:HD], in_=psr,
                        func=mybir.ActivationFunctionType.Identity,
                        bias=0.0, scale=1.0)
                    nc.vector.tensor_add(
                        v_sb[:, tb, :, 0:HD], v_sb[:, tb, :, 0:HD],
                        bv.rearrange("p (h d) -> p h d", h=NHL))
                else:
                    nc.vector.tensor_copy(v_sb[:, tb, :, 0:HD], psr)

            def av_items(hp, qt, exs):
                """Work-queue items for normalized attention of (hp, qt):
                one op-accumulation item per h01 (4 qq groups + rescale, so
                the op PSUM tile lives within a single item), then one
                transpose item. FIFO order keeps rescales before transposes."""
                nkb = 4 * qt + 4
                state = {}

                def havs(h01):
                    def run():
                        if "att" not in state:
                            state["att"] = attp.tile([128, 4, 2, HD], cdt,
                                                     name="att")
                        op = psp.tile([128, 4, 65], F32, tag="B",
                                      name="op", bufs=2)
                        h = 2 * hp + h01
                        for qq in range(4):
                            kbs = [kb for kb in range(nkb) if kb - 4 * qt <= qq]
                            for idx, kb in enumerate(kbs):
                                nc.tensor.matmul(
                                    op[:, qq, :],
                                    lhsT=exs[kb][:, h01, qq * 128:(qq + 1) * 128],
                                    rhs=v_sb[:, kb, h, :],
                                    start=(idx == 0), stop=(idx == len(kbs) - 1),
                                )
                        rc = small.tile([128, 4], F32, tag="rc")
                        nc.vector.reciprocal(rc[:], op[:, :, HD])
                        for q2 in range(4):
                            nc.vector.tensor_scalar_mul(
                                state["att"][:, q2, h01, :],
                                op[:, q2, 0:HD], rc[:, q2:q2 + 1])
                    return run

                def transp():
                    # DMA XBAR transpose SBUF->SBUF: frees the PE (and the
                    # psum tile + DVE copy) from the 128x128 transposes.
                    att = state["att"]
                    for qq in range(4):
                        qb = qt * 4 + qq
                        nc.sync.dma_start(
                            attT_sb[:, hp, qb * 128:(qb + 1) * 128],
                            att[:, qq, :, :].rearrange("p a b -> p (a b)"),
                            transpose=True)

                return [havs(0), havs(1), transp]

            def proj_items(qt):
                """Work-queue items: output projection + store, one per
                128-token block."""
                def one(tb):
                    def run():
                        pa = psp.tile([128, 512], F32, tag="B", name="pa",
                                      bufs=2)
                        pb = psp.tile([128, 256], F32, tag="B", name="pb",
                                      bufs=2)
                        for hp2 in range(3):
                            nc.tensor.matmul(
                                pa[:],
                                lhsT=attT_sb[:, hp2, tb * 128:(tb + 1) * 128],
                                rhs=wp[:, hp2, 0:512],
                                start=(hp2 == 0), stop=(hp2 == 2))
                            nc.tensor.matmul(
                                pb[:],
                                lhsT=attT_sb[:, hp2, tb * 128:(tb + 1) * 128],
                                rhs=wp[:, hp2, 512:768],
                                start=(hp2 == 0), stop=(hp2 == 2))
                        ysb = work.tile([128, 768], F32, tag="ysb")
                        nc.vector.tensor_copy(ysb[:, 0:512], pa[:])
                        nc.vector.tensor_copy(ysb[:, 512:768], pb[:])
                        nc.sync.dma_start(
                            y_d[tb * 128:(tb + 1) * 128, :], ysb[:])
                    return run
                return [one(tb) for tb in range(qt * 4, qt * 4 + 4)]

            xT_r = xT_d.rearrange("(n p) t -> p n t", p=128)
            wqk_r = wqk_d.rearrange("(n p) m -> p n m", p=128)
            wv_r = wv_d.rearrange("(n p) m -> p n m", p=128)
            wp_r = wp_d.rearrange("(n p) m -> p n m", p=128)

            def load_main_inputs():
                for kc in range(6):
                    nc.sync.dma_start(xT[:, kc, :], xT_r[:, kc, :])
                    nc.sync.dma_start(wqk[:, kc, :], wqk_r[:, kc, :])
                for kc in range(6):
                    nc.sync.dma_start(wv[:, kc, :], wv_r[:, kc, :])
                if with_bqk:
                    nc.sync.dma_start(bqk[:], bqk_d.rearrange("(n p) o -> p n o", p=128))
                if with_bv:
                    nc.sync.dma_start(bv[:], bv_d[:])

            def prologue():
                load_main_inputs()
                nc.sync.dma_start(mask[:], mask_d[:])
                nc.sync.dma_start(ident[:], id_d[:])
                nc.sync.dma_start(wp[:], wp_r)
                nc.gpsimd.memset(v_sb[:, :, :, HD:HD + 1], 1.0)

            def body(reload=False):
                from collections import deque

                # lead-in: q/k for head-pair 0, token chunk 0 only
                qk_nt(0, 0)
                qk_nt(3, 0)

                def filler(f):
                    def run():
                        if f[0] == "qk":
                            qk_nt(f[1], f[2])
                        elif f[0] == "v":
                            v_tb(f[1])
                        else:
                            load_main_inputs()
                    return run

                # Deferred PE work (QKV chunks, V blocks, AV groups,
                # transposes, projection blocks, input reload) is held in one
                # FIFO queue and drained a few items per kb-slot so the PE
                # always has issued-and-ready work while ACT grinds through
                # the exps. Each filler carries (due_slot, deadline_position):
                # it enters the queue at its due slot (shaping the interleave)
                # and the deadline forces a drain before the position whose
                # scores/AV read its output (program-order dependency).
                # dues[hp] = list of (slot_within_hp, deadline_pos, spec)
                dues = {0: [], 1: [], 2: []}
                d0 = [(1, 1, ("qk", 0, 1)), (1, 1, ("qk", 3, 1)),
                      (2, 1, ("v", 0)), (2, 1, ("v", 1)),
                      (3, 1, ("v", 2)), (3, 1, ("v", 3)),
                      (4, 2, ("qk", 0, 2)), (5, 2, ("qk", 3, 2)),
                      (6, 3, ("qk", 0, 3)), (7, 3, ("qk", 3, 3))]
                for k, tb in enumerate(range(4, NTB)):
                    d0.append((8 + k, tb // 4 + 1, ("v", tb)))
                for k, (j, nt) in enumerate([(j, nt) for j in (1, 4)
                                             for nt in range(4)]):
                    d0.append((20 + k, 4, ("qk", j, nt)))
                dues[0] = d0
                d1 = [(1 + 4 * k, 8, ("qk", j, nt))
                      for k, (j, nt) in enumerate([(j, nt) for j in (2, 5)
                                                   for nt in range(4)])]
                if reload:
                    # Re-issue next iteration's input loads once xT/wqk/wv
                    # are dead; the SP stream stays ahead of the y-output
                    # DMAs so the reload overlaps the attention tail.
                    d1.append((34, 12, ("reload",)))
                dues[1] = d1

                pending = deque()  # of (deadline_pos, closure)

                def pop_one():
                    pending.popleft()[1]()

                seq = [(hp, qt) for hp in range(3) for qt in range(4)]
                hp_slot = {0: 0, 1: 0, 2: 0}
                prev = None          # (hp, qt, exs) pending AV
                for i, (hp, qt) in enumerate(seq):
                    # drain everything the upcoming reads depend on
                    while any(dl <= i for dl, _ in pending):
                        pop_one()
                    if prev is not None:
                        phh, pqt, pexs = prev
                        for it in av_items(phh, pqt, pexs):
                            pending.append((i + 2, it))
                        if phh == 2:
                            for it in proj_items(pqt):
                                pending.append((i + 2, it))
                    nkb = 4 * qt + 4
                    nkb_next = 4 * seq[i + 1][1] + 4 if i + 1 < len(seq) else 0
                    exs = []
                    for kb in range(nkb):
                        diag = kb - 4 * qt
                        q_lo = diag * 128 if diag >= 0 else 0
                        sc = psp.tile([128, 2, 512], F32, tag="A", name="sc")
                        for h01 in range(2):
                            pb = h01 * 64
                            nc.tensor.matmul(
                                sc[:, h01, q_lo:512],
                                lhsT=kt_sb[pb:pb + 64, hp, kb * 128:(kb + 1) * 128],
                                rhs=qt_sb[pb:pb + 64, hp, qt * 512 + q_lo:(qt + 1) * 512],
                                start=True, stop=True,
                            )
                        ex = expool.tile([128, 2, 512], cdt, tag="ex", name="ex")
                        nc.scalar.activation(
                            out=ex[:, :, q_lo:512], in_=sc[:, :, q_lo:512],
                            func=mybir.ActivationFunctionType.Exp)
                        if diag >= 0:
                            for h01 in range(2):
                                nc.gpsimd.tensor_mul(
                                    ex[:, h01, q_lo:q_lo + 128],
                                    ex[:, h01, q_lo:q_lo + 128],
                                    mask[:])
                        exs.append(ex)
                        # enqueue fillers coming due at this slot
                        hp_slot[hp] += 1
                        for slot, dl, spec in dues[hp]:
                            if slot == hp_slot[hp]:
                                pending.append((dl, filler(spec)))
                        # drain paced to finish by the end of the next position
                        horizon = (nkb - 1 - kb) + nkb_next + 1
                        budget = -(-len(pending) // horizon)
                        for _ in range(min(budget, len(pending))):
                            pop_one()
                    prev = (hp, qt, exs)
                # tail: last AV + projection, then any stragglers
                phh, pqt, pexs = prev
                for it in av_items(phh, pqt, pexs):
                    pending.append((99, it))
                for it in proj_items(pqt):
                    pending.append((99, it))
                while pending:
                    pop_one()
                if reload:
                    # wp is only dead after the final proj; reload it behind
                    # the y-output DMAs so they are not held up.
                    nc.sync.dma_start(wp[:], wp_r)

            prologue()
            if n_iters == 1:
                body(reload=False)
            else:
                # unroll 2 bodies per hardware-loop iteration to halve the
                # loop back-edge cost (barrier + sequencer re-entry ~10us)
                n_loop, n_extra = divmod(n_iters, 2)
                with tc.For_i(0, n_loop, 1,
                              staggered_reset=True,
                              hint_engines=(mybir.EngineType.PE,
                                            mybir.EngineType.DVE,
                                            mybir.EngineType.Activation)):
                    body(reload=True)
                    body(reload=True)
                for _ in range(n_extra):
                    body(reload=True)

    nc.compile()
    return nc


def _host_prep(inputs, cdt_np):
    """Per-core input maps from full inputs."""
    x = np.asarray(inputs["x"], np.float32)
    w_attn = np.asarray(inputs["w_attn"], np.float32)
    b_attn = np.asarray(inputs["b_attn"], np.float32)
    w_proj = np.asarray(inputs["w_proj"], np.float32)

    mask = (np.arange(128)[:, None] <= np.arange(128)[None, :]).astype(cdt_np)
    ident = np.eye(128, dtype=cdt_np)
    with_bqk = bool(np.any(b_attn[0:1536] != 0))
    with_bv = bool(np.any(b_attn[1536:2304] != 0))

    in_maps = []
    for c in range(N_CORES):
        b, hg = c // 2, c % 2
        cols = slice(hg * 384, hg * 384 + 384)
        wq = w_attn[:, 0:768][:, cols] * 0.125
        wk = w_attn[:, 768:1536][:, cols]
        m = {
            "xT": np.ascontiguousarray(x[b].T).astype(cdt_np),
            "wqk": np.concatenate([wq, wk], axis=1).astype(cdt_np),
            "wv": np.ascontiguousarray(w_attn[:, 1536:2304][:, cols]).astype(cdt_np),
            "wp": np.ascontiguousarray(w_proj[cols, :]).astype(cdt_np),
            "mask": mask,
            "ident": ident,
        }
        if with_bqk:
            bq = b_attn[0:768][cols] * 0.125
            bk = b_attn[768:1536][cols]
            m["bqk"] = np.concatenate([bq, bk]).astype(np.float32).reshape(C, 1)
        if with_bv:
            bv = b_attn[1536:2304][cols].astype(cdt_np)
            m["bv"] = np.broadcast_to(bv, (128, 384)).copy()
        in_maps.append(m)
    return in_maps, with_bqk, with_bv


_CACHE = {}


def _get_runner(cdt, n_iters, with_bqk, with_bv, donate=True):
    """Build program + persistent jitted PJRT callable (cached)."""
    key = (str(cdt), n_iters, with_bqk, with_bv, donate)
    if key in _CACHE:
        return _CACHE[key]

    import jax
    from jax.sharding import Mesh, PartitionSpec
    from jax.experimental.shard_map import shard_map
    from concourse.bass2jax import (_bass_exec_p, install_neuronx_cc_hook,
                                    partition_id_tensor)

    nc = _build_program(cdt=cdt, n_iters=n_iters,
                        with_bqk=with_bqk, with_bv=with_bv)
    install_neuronx_cc_hook()

    partition_name = nc.partition_id_tensor.name if nc.partition_id_tensor else None
    in_names, out_names, out_avals = [], [], []
    for alloc in nc.m.functions[0].allocations:
        if not isinstance(alloc, mybir.MemoryLocationSet):
            continue
        name = alloc.memorylocations[0].name
        if alloc.kind == "ExternalInput":
            if name != partition_name:
                in_names.append(name)
        elif alloc.kind == "ExternalOutput":
            out_names.append(name)
            out_avals.append(jax.core.ShapedArray(
                tuple(alloc.tensor_shape), mybir.dt.np(alloc.dtype)))
    n_params = len(in_names)
    n_outs = len(out_avals)
    all_names = list(in_names) + list(out_names)
    if partition_name is not None:
        all_names.append(partition_name)
    donate_ = tuple(range(n_params, n_params + n_outs))

    def _bodyfn(*args):
        operands = list(args)
        if partition_name is not None:
            operands.append(partition_id_tensor())
        outs = _bass_exec_p.bind(
            *operands,
            out_avals=tuple(out_avals),
            in_names=tuple(all_names),
            out_names=tuple(out_names),
            lowering_input_output_aliases=(),
            sim_require_finite=True,
            sim_require_nnan=True,
            nc=nc,
        )
        return tuple(outs)

    devices = jax.devices()[:N_CORES]
    mesh = Mesh(np.asarray(devices), ("core",))
    in_specs = (PartitionSpec("core"),) * (n_params + n_outs)
    out_specs = (PartitionSpec("core"),) * n_outs
    fn = jax.jit(
        shard_map(_bodyfn, mesh=mesh, in_specs=in_specs, out_specs=out_specs,
                  check_rep=False),
        donate_argnums=donate_ if donate else (), keep_unused=True)

    runner = (fn, in_names, out_names, out_avals)
    _CACHE[key] = runner
    return runner


def _run(in_maps, cdt, n_iters, with_bqk, with_bv):
    import jax
    fn, in_names, out_names, out_avals = _get_runner(cdt, n_iters, with_bqk, with_bv)
    concat_in = [np.concatenate([m[nm] for m in in_maps], axis=0)
                 for nm in in_names]
    zeros = [np.zeros((N_CORES * av.shape[0], *av.shape[1:]), av.dtype)
             for av in out_avals]
    outs = fn(*concat_in, *zeros)
    jax.block_until_ready(outs)
    y = np.asarray(outs[out_names.index("y")]).reshape(N_CORES, T, C)
    return y


def kernel(**inputs) -> np.ndarray:
    import ml_dtypes
    cdt, cdt_np = mybir.dt.bfloat16, ml_dtypes.bfloat16
    in_maps, with_bqk, with_bv = _host_prep(inputs, cdt_np)
    y_parts = _run(in_maps, cdt, 1, with_bqk, with_bv)

    b_proj = np.asarray(inputs["b_proj"], np.float32)
    out = np.empty((B, T, C), np.float32)
    for b in range(B):
        out[b] = y_parts[2 * b] + y_parts[2 * b + 1] + b_proj
    return out


# revision 19
# speedup vs baseline: 1.2807x; 1.0572x over previous
"""Causal self-attention (B=4, T=2048, C=768, 12 heads) on 8 Trainium2 NeuronCores.

Sharding: core c -> batch b = c//2, head group hg = c%2 (6 heads each).
Each core computes QKV for its (b, 6 heads), flash-style causal attention in a
transposed layout (scores^T = K^T-chunk @ Q, exp on ACT, ones-column in V for
row sums via the PE), transposes the per-head attention output on the PE, and
applies its slice of the output projection. Host sums the two half-head partial
y's per batch and adds b_proj.

Schedule: hp-major attention pipeline. Attention scores for head-pair hp start
as soon as q/k for hp0 token-chunk 0 exist; the remaining QKV matmuls (other
q/k chunks + all of V) are issued as PE fillers interleaved into the EXP-paced
scores loop so the PE never starves while ACT works through the exps. AV for
(hp,qt) lags one step behind scores (lag-1 software pipeline across hp
boundaries); projection chases AV during hp2. Diagonal-supertile score matmuls
are trimmed to the causal region.
"""

import numpy as np

import concourse.bacc as bacc
import concourse.bass as bass
import concourse.mybir as mybir
import concourse.tile as tile

# problem constants (hardcoded per harness contract)
B, T, C = 4, 2048, 768
NH, HD = 12, 64
NHL = 6            # heads per core
N_CORES = 8
TQ = 512           # q supertile width
NTB = T // 128     # 16 token blocks

F32 = mybir.dt.float32


def _build_program(cdt=mybir.dt.bfloat16, n_iters=1, with_bqk=False, with_bv=False):
    """Build the SPMD single-core program. Returns nc."""
    nc = bacc.Bacc("TRN2", target_bir_lowering=False, debug=False,
                   num_devices=N_CORES)

    xT_d = nc.dram_tensor("xT", [C, T], cdt, kind="ExternalInput")
    wqk_d = nc.dram_tensor("wqk", [C, 768], cdt, kind="ExternalInput")
    wv_d = nc.dram_tensor("wv", [C, 384], cdt, kind="ExternalInput")
    wp_d = nc.dram_tensor("wp", [384, C], cdt, kind="ExternalInput")
    mask_d = nc.dram_tensor("mask", [128, 128], cdt, kind="ExternalInput")
    id_d = nc.dram_tensor("ident", [128, 128], cdt, kind="ExternalInput")
    if with_bqk:
        bqk_d = nc.dram_tensor("bqk", [C, 1], F32, kind="ExternalInput")
    if with_bv:
        bv_d = nc.dram_tensor("bv", [128, 384], cdt, kind="ExternalInput")
    y_d = nc.dram_tensor("y", [T, C], F32, kind="ExternalOutput")

    with tile.TileContext(nc) as tc:
        with (
            tc.tile_pool(name="consts", bufs=1) as consts,
            tc.tile_pool(name="big", bufs=1) as big,
            tc.tile_pool(name="work", bufs=3) as work,
            tc.tile_pool(name="expool", bufs=34) as expool,
            tc.tile_pool(name="attp", bufs=3) as attp,
            tc.tile_pool(name="small", bufs=8) as small,
            tc.tile_pool(name="ps", bufs=3, space="PSUM") as psp,
        ):
            xT = consts.tile([128, 6, T], cdt)
            wqk = consts.tile([128, 6, 768], cdt)
            wv = consts.tile([128, 6, 384], cdt)
            wp = consts.tile([128, 3, 768], cdt)
            mask = consts.tile([128, 128], cdt)
            ident = consts.tile([128, 128], cdt)
            if with_bqk:
                bqk = consts.tile([128, 6, 1], F32)
            if with_bv:
                bv = consts.tile([128, 384], cdt)

            qt_sb = big.tile([128, 3, T], cdt)
            kt_sb = big.tile([128, 3, T], cdt)
            v_sb = big.tile([128, NTB, NHL, 65], cdt)
            attT_sb = big.tile([128, 3, T], cdt)

            def qk_nt(j, nt):
                """One 512-token chunk of q (j<3) / k (j>=3), partition chunk j%3."""
                dst = qt_sb if j < 3 else kt_sb
                jj = j % 3
                ps = psp.tile([128, 512], F32, tag="B", name="qk", bufs=2)
                for kc in range(6):
                    nc.tensor.matmul(
                        ps[:],
                        lhsT=wqk[:, kc, j * 128:(j + 1) * 128],
                        rhs=xT[:, kc, nt * 512:(nt + 1) * 512],
                        start=(kc == 0), stop=(kc == 5),
                    )
                if with_bqk:
                    nc.vector.tensor_scalar_add(
                        dst[:, jj, nt * 512:(nt + 1) * 512], ps[:],
                        bqk[:, jj if j < 3 else jj + 3])
                else:
                    nc.vector.tensor_copy(
                        dst[:, jj, nt * 512:(nt + 1) * 512], ps[:])

            def v_tb(tb):
                """V projection for one 128-token block, natural layout."""
                ps = psp.tile([128, 384], F32, tag="B", name="psv", bufs=2)
                for kc in range(6):
                    nc.tensor.matmul(
                        ps[:],
                        lhsT=xT[:, kc, tb * 128:(tb + 1) * 128],
                        rhs=wv[:, kc, :],
                        start=(kc == 0), stop=(kc == 5),
                    )
                psr = ps[:].rearrange("p (h d) -> p h d", h=NHL)
                if with_bv:
                    nc.scalar.activation(
                        out=v_sb[:, tb, :, 0:HD], in_=psr,
                        func=mybir.ActivationFunctionType.Identity,
                        bias=0.0, scale=1.0)
                    nc.vector.tensor_add(
                        v_sb[:, tb, :, 0:HD], v_sb[:, tb, :, 0:HD],
                        bv.rearrange("p (h d) -> p h d", h=NHL))
                else:
                    nc.vector.tensor_copy(v_sb[:, tb, :, 0:HD], psr)

            def av_items(hp, qt, exs):
                """Work-queue items for normalized attention of (hp, qt):
                one op-accumulation item per h01 (4 qq groups + rescale, so
                the op PSUM tile lives within a single item), then one
                transpose item. FIFO order keeps rescales before transposes."""
                nkb = 4 * qt + 4
                state = {}

                def havs(h01):
                    def run():
                        if "att" not in state:
                            state["att"] = attp.tile([128, 4, 2, HD], cdt,
                                                     name="att")
                        op = psp.tile([128, 4, 65], F32, tag="B",
                                      name="op", bufs=2)
                        h = 2 * hp + h01
                        for qq in range(4):
                            kbs = [kb for kb in range(nkb) if kb - 4 * qt <= qq]
                            for idx, kb in enumerate(kbs):
                                nc.tensor.matmul(
                                    op[:, qq, :],
                                    lhsT=exs[kb][:, h01, qq * 128:(qq + 1) * 128],
                                    rhs=v_sb[:, kb, h, :],
                                    start=(idx == 0), stop=(idx == len(kbs) - 1),
                                )
                        rc = small.tile([128, 4], F32, tag="rc")
                        nc.vector.reciprocal(rc[:], op[:, :, HD])
                        for q2 in range(4):
                            nc.vector.tensor_scalar_mul(
                                state["att"][:, q2, h01, :],
                                op[:, q2, 0:HD], rc[:, q2:q2 + 1])
                    return run

                def transp():
                    # hp0/hp1: DMA XBAR transpose SBUF->SBUF frees the PE
                    # (and psum + DVE copy); its ~1.5us latency is hidden,
                    # since proj only reads attT much later. hp2 feeds proj
                    # immediately, so keep the low-latency PE transpose there.
                    att = state["att"]
                    for qq in range(4):
                        qb = qt * 4 + qq
                        if hp < 2:
                            nc.sync.dma_start(
                                attT_sb[:, hp, qb * 128:(qb + 1) * 128],
                                att[:, qq, :, :].rearrange("p a b -> p (a b)"),
                                transpose=True)
                        else:
                            tp = psp.tile([128, 4, 65], cdt, tag="B",
                                          name="tp", bufs=2)
                            tpa = tp.rearrange("p a b -> p (a b)")[:, 0:128]
                            nc.tensor.transpose(tpa, att[:, qq, :, :], ident[:])
                            nc.vector.tensor_copy(
                                attT_sb[:, hp, qb * 128:(qb + 1) * 128], tpa)

                return [havs(0), havs(1), transp]

            def proj_items(qt):
                """Work-queue items: output projection + store, one per
                128-token block."""
                def one(tb):
                    def run():
                        pa = psp.tile([128, 512], F32, tag="B", name="pa",
                                      bufs=2)
                        pb = psp.tile([128, 256], F32, tag="B", name="pb",
                                      bufs=2)
                        for hp2 in range(3):
                            nc.tensor.matmul(
                                pa[:],
                                lhsT=attT_sb[:, hp2, tb * 128:(tb + 1) * 128],
                                rhs=wp[:, hp2, 0:512],
                                start=(hp2 == 0), stop=(hp2 == 2))
                            nc.tensor.matmul(
                                pb[:],
                                lhsT=attT_sb[:, hp2, tb * 128:(tb + 1) * 128],
                                rhs=wp[:, hp2, 512:768],
                                start=(hp2 == 0), stop=(hp2 == 2))
                        ysb = work.tile([128, 768], F32, tag="ysb")
                        nc.vector.tensor_copy(ysb[:, 0:512], pa[:])
                        nc.vector.tensor_copy(ysb[:, 512:768], pb[:])
                        nc.sync.dma_start(
                            y_d[tb * 128:(tb + 1) * 128, :], ysb[:])
                    return run
                return [one(tb) for tb in range(qt * 4, qt * 4 + 4)]

            xT_r = xT_d.rearrange("(n p) t -> p n t", p=128)
            wqk_r = wqk_d.rearrange("(n p) m -> p n m", p=128)
            wv_r = wv_d.rearrange("(n p) m -> p n m", p=128)
            wp_r = wp_d.rearrange("(n p) m -> p n m", p=128)

            def load_main_inputs():
                for kc in range(6):
                    nc.sync.dma_start(xT[:, kc, :], xT_r[:, kc, :])
                    nc.sync.dma_start(wqk[:, kc, :], wqk_r[:, kc, :])
                for kc in range(6):
                    nc.sync.dma_start(wv[:, kc, :], wv_r[:, kc, :])
                if with_bqk:
                    nc.sync.dma_start(bqk[:], bqk_d.rearrange("(n p) o -> p n o", p=128))
                if with_bv:
                    nc.sync.dma_start(bv[:], bv_d[:])

            def prologue():
                load_main_inputs()
                nc.sync.dma_start(mask[:], mask_d[:])
                nc.sync.dma_start(ident[:], id_d[:])
                nc.sync.dma_start(wp[:], wp_r)
                nc.gpsimd.memset(v_sb[:, :, :, HD:HD + 1], 1.0)

            def body(reload=False):
                from collections import deque

                # lead-in: q/k for head-pair 0, token chunk 0 only
                qk_nt(0, 0)
                qk_nt(3, 0)

                def filler(f):
                    def run():
                        if f[0] == "qk":
                            qk_nt(f[1], f[2])
                        elif f[0] == "v":
                            v_tb(f[1])
                        else:
                            load_main_inputs()
                    return run

                # Deferred PE work (QKV chunks, V blocks, AV groups,
                # transposes, projection blocks, input reload) is held in one
                # FIFO queue and drained a few items per kb-slot so the PE
                # always has issued-and-ready work while ACT grinds through
                # the exps. Each filler carries (due_slot, deadline_position):
                # it enters the queue at its due slot (shaping the interleave)
                # and the deadline forces a drain before the position whose
                # scores/AV read its output (program-order dependency).
                # dues[hp] = list of (slot_within_hp, deadline_pos, spec)
                dues = {0: [], 1: [], 2: []}
                d0 = [(1, 1, ("qk", 0, 1)), (1, 1, ("qk", 3, 1)),
                      (2, 1, ("v", 0)), (2, 1, ("v", 1)),
                      (3, 1, ("v", 2)), (3, 1, ("v", 3)),
                      (4, 2, ("qk", 0, 2)), (5, 2, ("qk", 3, 2)),
                      (6, 3, ("qk", 0, 3)), (7, 3, ("qk", 3, 3))]
                for k, tb in enumerate(range(4, NTB)):
                    d0.append((8 + k, tb // 4 + 1, ("v", tb)))
                for k, (j, nt) in enumerate([(j, nt) for j in (1, 4)
                                             for nt in range(4)]):
                    d0.append((20 + k, 4, ("qk", j, nt)))
                dues[0] = d0
                d1 = [(1 + 4 * k, 8, ("qk", j, nt))
                      for k, (j, nt) in enumerate([(j, nt) for j in (2, 5)
                                                   for nt in range(4)])]
                if reload:
                    # Re-issue next iteration's input loads once xT/wqk/wv
                    # are dead; the SP stream stays ahead of the y-output
                    # DMAs so the reload overlaps the attention tail.
                    d1.append((34, 12, ("reload",)))
                dues[1] = d1

                pending = deque()  # of (deadline_pos, closure)

                def pop_one():
                    pending.popleft()[1]()

                seq = [(hp, qt) for hp in range(3) for qt in range(4)]
                hp_slot = {0: 0, 1: 0, 2: 0}
                prev = None          # (hp, qt, exs) pending AV
                for i, (hp, qt) in enumerate(seq):
                    # drain everything the upcoming reads depend on
                    while any(dl <= i for dl, _ in pending):
                        pop_one()
                    if prev is not None:
                        phh, pqt, pexs = prev
                        for it in av_items(phh, pqt, pexs):
                            pending.append((i + 2, it))
                        if phh == 2:
                            for it in proj_items(pqt):
                                pending.append((i + 2, it))
                    nkb = 4 * qt + 4
                    nkb_next = 4 * seq[i + 1][1] + 4 if i + 1 < len(seq) else 0
                    exs = []
                    for kb in range(nkb):
                        diag = kb - 4 * qt
                        q_lo = diag * 128 if diag >= 0 else 0
                        sc = psp.tile([128, 2, 512], F32, tag="A", name="sc")
                        for h01 in range(2):
                            pb = h01 * 64
                            nc.tensor.matmul(
                                sc[:, h01, q_lo:512],
                                lhsT=kt_sb[pb:pb + 64, hp, kb * 128:(kb + 1) * 128],
                                rhs=qt_sb[pb:pb + 64, hp, qt * 512 + q_lo:(qt + 1) * 512],
                                start=True, stop=True,
                            )
                        ex = expool.tile([128, 2, 512], cdt, tag="ex", name="ex")
                        nc.scalar.activation(
                            out=ex[:, :, q_lo:512], in_=sc[:, :, q_lo:512],
                            func=mybir.ActivationFunctionType.Exp)
                        if diag >= 0:
                            for h01 in range(2):
                                nc.gpsimd.tensor_mul(
                                    ex[:, h01, q_lo:q_lo + 128],
                                    ex[:, h01, q_lo:q_lo + 128],
                                    mask[:])
                        exs.append(ex)
                        # enqueue fillers coming due at this slot
                        hp_slot[hp] += 1
                        for slot, dl, spec in dues[hp]:
                            if slot == hp_slot[hp]:
                                pending.append((dl, filler(spec)))
                        # drain paced to finish by the end of the next position
                        horizon = (nkb - 1 - kb) + nkb_next + 1
                        budget = -(-len(pending) // horizon)
                        for _ in range(min(budget, len(pending))):
                            pop_one()
                    prev = (hp, qt, exs)
                # tail: last AV + projection, then any stragglers
                phh, pqt, pexs = prev
                for it in av_items(phh, pqt, pexs):
                    pending.append((99, it))
                for it in proj_items(pqt):
                    pending.append((99, it))
                while pending:
                    pop_one()
                if reload:
                    # wp is only dead after the final proj; reload it behind
                    # the y-output DMAs so they are not held up.
                    nc.sync.dma_start(wp[:], wp_r)

            prologue()
            if n_iters == 1:
                body(reload=False)
            else:
                # unroll 2 bodies per hardware-loop iteration to halve the
                # loop back-edge cost (barrier + sequencer re-entry ~10us)
                n_loop, n_extra = divmod(n_iters, 2)
                with tc.For_i(0, n_loop, 1,
                              staggered_reset=True,
                              hint_engines=(mybir.EngineType.PE,
                                            mybir.EngineType.DVE,
                                            mybir.EngineType.Activation)):
                    body(reload=True)
                    body(reload=True)
                for _ in range(n_extra):
                    body(reload=True)

    nc.compile()
    return nc


def _host_prep(inputs, cdt_np):
    """Per-core input maps from full inputs."""
    x = np.asarray(inputs["x"], np.float32)
    w_attn = np.asarray(inputs["w_attn"], np.float32)
    b_attn = np.asarray(inputs["b_attn"], np.float32)
    w_proj = np.asarray(inputs["w_proj"], np.float32)

    mask = (np.arange(128)[:, None] <= np.arange(128)[None, :]).astype(cdt_np)
    ident = np.eye(128, dtype=cdt_np)
    with_bqk = bool(np.any(b_attn[0:1536] != 0))
    with_bv = bool(np.any(b_attn[1536:2304] != 0))

    in_maps = []
    for c in range(N_CORES):
        b, hg = c // 2, c % 2
        cols = slice(hg * 384, hg * 384 + 384)
        wq = w_attn[:, 0:768][:, cols] * 0.125
        wk = w_attn[:, 768:1536][:, cols]
        m = {
            "xT": np.ascontiguousarray(x[b].T).astype(cdt_np),
            "wqk": np.concatenate([wq, wk], axis=1).astype(cdt_np),
            "wv": np.ascontiguousarray(w_attn[:, 1536:2304][:, cols]).astype(cdt_np),
            "wp": np.ascontiguousarray(w_proj[cols, :]).astype(cdt_np),
            "mask": mask,
            "ident": ident,
        }
        if with_bqk:
            bq = b_attn[0:768][cols] * 0.125
            bk = b_attn[768:1536][cols]
            m["bqk"] = np.concatenate([bq, bk]).astype(np.float32).reshape(C, 1)
        if with_bv:
            bv = b_attn[1536:2304][cols].astype(cdt_np)
            m["bv"] = np.broadcast_to(bv, (128, 384)).copy()
        in_maps.append(m)
    return in_maps, with_bqk, with_bv


_CACHE = {}


def _get_runner(cdt, n_iters, with_bqk, with_bv, donate=True):
    """Build program + persistent jitted PJRT callable (cached)."""
    key = (str(cdt), n_iters, with_bqk, with_bv, donate)
    if key in _CACHE:
        return _CACHE[key]

    import jax
    from jax.sharding import Mesh, PartitionSpec
    from jax.experimental.shard_map import shard_map
    from concourse.bass2jax import (_bass_exec_p, install_neuronx_cc_hook,
                                    partition_id_tensor)

    nc = _build_program(cdt=cdt, n_iters=n_iters,
                        with_bqk=with_bqk, with_bv=with_bv)
    install_neuronx_cc_hook()

    partition_name = nc.partition_id_tensor.name if nc.partition_id_tensor else None
    in_names, out_names, out_avals = [], [], []
    for alloc in nc.m.functions[0].allocations:
        if not isinstance(alloc, mybir.MemoryLocationSet):
            continue
        name = alloc.memorylocations[0].name
        if alloc.kind == "ExternalInput":
            if name != partition_name:
                in_names.append(name)
        elif alloc.kind == "ExternalOutput":
            out_names.append(name)
            out_avals.append(jax.core.ShapedArray(
                tuple(alloc.tensor_shape), mybir.dt.np(alloc.dtype)))
    n_params = len(in_names)
    n_outs = len(out_avals)
    all_names = list(in_names) + list(out_names)
    if partition_name is not None:
        all_names.append(partition_name)
    donate_ = tuple(range(n_params, n_params + n_outs))

    def _bodyfn(*args):
        operands = list(args)
        if partition_name is not None:
            operands.append(partition_id_tensor())
        outs = _bass_exec_p.bind(
            *operands,
            out_avals=tuple(out_avals),
            in_names=tuple(all_names),
            out_names=tuple(out_names),
            lowering_input_output_aliases=(),
            sim_require_finite=True,
            sim_require_nnan=True,
            nc=nc,
        )
        return tuple(outs)

    devices = jax.devices()[:N_CORES]
    mesh = Mesh(np.asarray(devices), ("core",))
    in_specs = (PartitionSpec("core"),) * (n_params + n_outs)
    out_specs = (PartitionSpec("core"),) * n_outs
    fn = jax.jit(
        shard_map(_bodyfn, mesh=mesh, in_specs=in_specs, out_specs=out_specs,
                  check_rep=False),
        donate_argnums=donate_ if donate else (), keep_unused=True)

    runner = (fn, in_names, out_names, out_avals)
    _CACHE[key] = runner
    return runner


def _run(in_maps, cdt, n_iters, with_bqk, with_bv):
    import jax
    fn, in_names, out_names, out_avals = _get_runner(cdt, n_iters, with_bqk, with_bv)
    concat_in = [np.concatenate([m[nm] for m in in_maps], axis=0)
                 for nm in in_names]
    zeros = [np.zeros((N_CORES * av.shape[0], *av.shape[1:]), av.dtype)
             for av in out_avals]
    outs = fn(*concat_in, *zeros)
    jax.block_until_ready(outs)
    y = np.asarray(outs[out_names.index("y")]).reshape(N_CORES, T, C)
    return y


def kernel(**inputs) -> np.ndarray:
    import ml_dtypes
    cdt, cdt_np = mybir.dt.bfloat16, ml_dtypes.bfloat16
    in_maps, with_bqk, with_bv = _host_prep(inputs, cdt_np)
    y_parts = _run(in_maps, cdt, 1, with_bqk, with_bv)

    b_proj = np.asarray(inputs["b_proj"], np.float32)
    out = np.empty((B, T, C), np.float32)
    for b in range(B):
        out[b] = y_parts[2 * b] + y_parts[2 * b + 1] + b_proj
    return out
